# revision 52
# baseline (speedup 1.0000x reference)
"""Trainium2 Bass kernel for nn_CrossLayer (dense transformer layer), v5.

Sharding: sequence-parallel over 8 cores (2 samples x 4 token-chunks of 512).
Each core computes its 512 token rows through CA -> SA -> FFN.

- quad layout for q/k: head-quad tiles [128, 2, T] with partition
  p -> head 4g+(p//32), dim d = 32*j + p%32.  Rope's rotate-half becomes
  a free-dim (j) swap (no PE rotation matmul, no ACT rot copy); the cos/sin
  tables carry the qn/8 per-channel factors (host-folded).  rms-norm
  Ln/Exp batched per quad ([4,T] ACT ops).
- scores are DoubleRow matmuls on [32, 2, .] slices (explicit
  tile_position for the base-96 head); AV keeps the ones-column denominator
  trick.  Attention is fp8 e4m3 end to end; weights pre-scaled x8; /8s
  fold into the tables and the softmax-denominator exp bias (-ln 64).
- PSUM tags: "big" [128,2,512] bufs=3 (6 banks; projections, scores, FFN
  p1/p3 pairs) + "px" (2 banks, AV accumulate).  Scores triple-buffer
  against the softmax EXP on ACT so PE and ACT overlap.
- CA quad-0 k/v is computed replicated (every core does all 2048 src
  tokens) because the collective cores take ~60us to initialize after NEFF
  start; quads 1-3 AllGather in pipelined chunks that land during earlier
  groups' attention.  SA gathers all four quads (CC warm by then).
- FFN: bf16 (fp8 gives no matmul throughput on this HW and costs 1.1e-2+
  rel err); streams W1/W3 in 2-chunk tiles, hT double-buffered; the last
  quarter's residual chunks stream out as they finish.
- xT = xraw * 1/denom multiplies run on the otherwise-idle GpSimd engine.
"""

import math
import sys
import types

import numpy as np
import ml_dtypes

B, N, DIM, HID, H, D = 2, 2048, 1024, 4096, 16, 64
TOK = 512  # tokens per core
NCORES = 8
EPS = 1e-6
THETA = 10000.0
P = 128
KO = DIM // P  # 8 contraction chunks
KOP = KO // 2  # 4 DoubleRow pair-chunks
HH = H // 2  # 8 head pairs
HC = HID // P  # 32 hidden chunks
NR = 4  # ranks per replica group
SRCN = 2048  # gathered kv tokens
SKC = SRCN // P  # 16 key chunks of 128 tokens
VW = D + 1  # v columns + ones column
HG = 4  # head groups (quads: 4 heads each)
WS = 8.0  # attention weight pre-scale for fp8
LN64 = math.log(64.0)

BF = ml_dtypes.bfloat16
F8 = ml_dtypes.float8_e4m3

_cache = {}


def _quad_perm():
    """New output-channel order o' = 256*g + 128*j + p for quad layout:
    orig channel c = 64*(4g + p//32) + 32*j + (p%32)."""
    perm = np.empty(DIM, np.int64)
    for g in range(4):
        for j in range(2):
            for p in range(P):
                perm[256 * g + 128 * j + p] = (
                    64 * (4 * g + p // 32) + 32 * j + (p % 32)
                )
    return perm


_QPERM = _quad_perm()


def _lhsT_dr(W):
    """[K, M] -> [M//128, 128(K%128), K//256, 2, 128(M%128)] fp8 x8.
    Slice [mt][:, kp] is a DoubleRow lhsT [128, 2, 128]."""
    K, M = W.shape
    A = W.reshape(K // 256, 2, P, M // P, P)
    return (A.transpose(3, 2, 0, 1, 4) * WS).astype(F8).copy()


def _rhs_dr(W):
    """[K, M] -> [128, K//256, 2, M] fp8 x8 rhs-style DoubleRow moving."""
    K, M = W.shape
    A = W.reshape(K // 256, 2, P, M)
    return (A.transpose(2, 0, 1, 3) * WS).astype(F8).copy()


def _lhsT_bf(W):
    """[K, M] -> [M//128, 128(K%128), K//128, 128(M%128)] bf16 (unscaled)."""
    K, M = W.shape
    return W.reshape(K // P, P, M // P, P).transpose(2, 1, 0, 3).astype(BF).copy()


def _featmajor(x, dt):
    """[tok, dim] -> [128, dim//128, tok]."""
    n = x.shape[0]
    return x.T.reshape(DIM // P, P, n).transpose(1, 0, 2).astype(dt).copy()


def _rope_tables_quad(pos, nv):
    """pos [n] int32, nv [64] norm weights -> (cos2, sinpm) [128, 2, n] bf16.
    cos2[p,j,t] = cos(pos_t * invf[p%32]) * nv[32j + p%32] / 8
    sinpm[p,0,t] = -sin(.) * nv[32 + p%32] / 8   (d0[:,j] = v1[:,1-j]*sinpm[:,j])
    sinpm[p,1,t] = +sin(.) * nv[p%32] / 8
    """
    n = pos.shape[0]
    invf = 1.0 / (THETA ** (np.arange(0, D, 2, dtype=np.float64) / D))  # [32]
    pm32 = np.tile(np.arange(32), 4)  # p % 32 for p in 0..127
    ang = pos.astype(np.float64)[None, :] * invf[pm32][:, None]  # [128, n]
    c = np.cos(ang)
    s = np.sin(ang)
    nv = np.asarray(nv, np.float64)
    cos2 = np.empty((P, 2, n), np.float64)
    sinpm = np.empty((P, 2, n), np.float64)
    cos2[:, 0, :] = c * (nv[pm32] / WS)[:, None]
    cos2[:, 1, :] = c * (nv[32 + pm32] / WS)[:, None]
    sinpm[:, 0, :] = -s * (nv[32 + pm32] / WS)[:, None]
    sinpm[:, 1, :] = s * (nv[pm32] / WS)[:, None]
    return cos2.astype(BF).copy(), sinpm.astype(BF).copy()


def _install_ntff_hook():
    try:
        from trn_agent_boot.trn_boot import _ntff_profile_via_ctypes
    except ImportError:
        return
    if "antenv.axon_hooks" in sys.modules:
        return
    try:
        hook = _ntff_profile_via_ctypes("/opt/axon/libaxon_pjrt.so")
    except OSError:
        return
    mod = types.ModuleType("antenv.axon_hooks")
    mod.get_axon_ntff_profile_hook = lambda: hook
    mod.set_axon_ntff_profile_hook = lambda h: None
    sys.modules["antenv.axon_hooks"] = mod
    import antenv

    antenv.axon_hooks = mod


def _split_multiwait(nc):
    """This walrus only supports one sync-wait on CTRL-encoded instructions
    (Drain/NoOp); hoist excess waits onto single-wait NoOps placed before."""
    from concourse import mybir

    n_split = 0
    for f in nc.m.functions:
        for bb in f.blocks:
            new = []
            changed = False
            for ins in bb.instructions:
                si = ins.sync_info
                if (
                    si is not None
                    and si.on_wait is not None
                    and len(si.on_wait) > 1
                ):
                    waits = list(si.on_wait)
                    keep, rest = waits[:1], waits[1:]
                    for k, w in enumerate(rest):
                        new.append(
                            mybir.InstNoOp(
                                name=f"{ins.name}-wsplit{k}",
                                engine=ins.engine,
                                sync_info=mybir.SyncInfo(
                                    on_wait=[w], on_update=[]
                                ),
                                bass_nofuse=True,
                            )
                        )
                    si.on_wait = keep
                    n_split += 1
                    changed = True
                new.append(ins)
            if changed:
                bb.instructions = new
    return n_split


def _build_bass():
    from contextlib import ExitStack

    import concourse.bass as bass
    import concourse.tile as tile
    from concourse import mybir

    f32 = mybir.dt.float32
    bf16 = mybir.dt.bfloat16
    fp8 = mybir.dt.float8e4
    u8 = mybir.dt.uint8
    AF = mybir.ActivationFunctionType
    DR = mybir.MatmulPerfMode.DoubleRow
    MUL = mybir.AluOpType.mult
    ADD = mybir.AluOpType.add
    # Schraudolph-style exp straight to fp8 e4m3 bits:
    # bits = round(score * A8/8 + (56 - C8 - 3*A8)); uint8 convert saturates
    # negatives to 0.  Softmax-level error matches exact-exp + fp8 rounding.
    A8 = 8.0 / math.log(2.0)
    EXP_S1 = A8 / 8.0
    EXP_S2 = 56.0 - 0.8 - 3.0 * A8
    # DVE bit-trick exp offload disabled: a run with it produced a one-off
    # NaN (suspected dep-tracking race on the bitcast write); the ~20us gain
    # is within run noise and not worth the correctness risk.
    DVE_EXP_KC = frozenset()

    nc = bass.Bass(num_devices=NCORES)

    def inp(name, shape, dt=fp8):
        return nc.dram_tensor(name, shape, dt, kind="ExternalInput")

    tgtT = inp("tgtT", [P, KO, TOK], f32)
    srcT = inp("srcT", [P, KO, SRCN])  # full sample (CA quad-0 kv replicated)
    srcTm = inp("srcTm", [P, KO, TOK])  # this core's 512-row src shard
    # rope/norm tables: [P, 2, TOK] bf16 per (pos-set, norm-vec);
    # ckca_f covers all SRCN src positions (streamed, for replicated quad 0)
    tab_names = ["cqca", "ckca", "cqsa", "cksa"]
    tabs_in = {}
    for tn in tab_names:
        tabs_in[tn] = (
            inp(tn + "_c", [P, 2, TOK], bf16),
            inp(tn + "_s", [P, 2, TOK], bf16),
        )
    ckf_c = inp("ckf_c", [P, 2, SRCN], bf16)
    ckf_s = inp("ckf_s", [P, 2, SRCN], bf16)
    caWq = inp("caWq", [HH, P, KOP, 2, P])
    caWk = inp("caWk", [HH, P, KOP, 2, P])
    caWv = inp("caWv", [P, KOP, 2, DIM])
    caWo = inp("caWo", [KO, P, KOP, 2, P])
    saWq = inp("saWq", [HH, P, KOP, 2, P])
    saWk = inp("saWk", [HH, P, KOP, 2, P])
    saWv = inp("saWv", [P, KOP, 2, DIM])
    saWo = inp("saWo", [KO, P, KOP, 2, P])
    W1i = inp("W1", [HC, P, KO, P], bf16)
    W3i = inp("W3", [HC, P, KO, P], bf16)
    W2i = inp("W2", [KO, P, HC, P], bf16)
    blk4 = inp("blk4", [P, 4], bf16)  # per-head ssq lhsT (block ones)
    mask4 = inp("mask4", [4, P], bf16)  # rsqrt bcast lhsT (block ones)
    ones_c = inp("ones_c", [P, 1], bf16)  # y-norm ssq lhsT
    ones_r128 = inp("ones_r128", [1, P], bf16)  # y-norm bcast lhsT

    outT = nc.dram_tensor("outT", [P, KO, TOK], f32, kind="ExternalOutput")

    groups = [[0, 1, 2, 3], [4, 5, 6, 7]]
    KSZ = P * 2 * TOK  # k fp8 words per rank per head-group
    VSZ = P * 4 * 4 * VW  # v fp8 words per rank per head-group

    with tile.TileContext(nc) as tc:
        ctx = ExitStack()
        with ctx:
            sing = ctx.enter_context(tc.tile_pool(name="sing", bufs=1))
            big = ctx.enter_context(tc.tile_pool(name="big", bufs=1))
            wpool = ctx.enter_context(tc.tile_pool(name="wpool", bufs=2))
            w13p = ctx.enter_context(tc.tile_pool(name="w13p", bufs=2))
            w2p = ctx.enter_context(tc.tile_pool(name="w2p", bufs=1))
            htp = ctx.enter_context(tc.tile_pool(name="htp", bufs=2))
            work = ctx.enter_context(tc.tile_pool(name="work", bufs=2))
            probp = ctx.enter_context(tc.tile_pool(name="probp", bufs=2))
            stat = ctx.enter_context(tc.tile_pool(name="stat", bufs=2))
            dram = ctx.enter_context(
                tc.tile_pool(name="dram", bufs=1, space="DRAM")
            )
            # PSUM: "big" [128,2,512] bufs=3 (6 banks) + "px" (2 banks)
            psum = ctx.enter_context(tc.tile_pool(name="psum", bufs=3, space="PSUM"))

            def big_ps(name):
                return psum.tile([P, 2, TOK], f32, tag="big", name=name)

            def small_ps(name, part=P):
                t = psum.tile([part, 2, TOK], f32, tag="big", name=name)
                return t

            # ---- resident tiles (DMA order: CA-kv critical path first)
            blk4_sb = sing.tile([P, 4], bf16)
            nc.sync.dma_start(blk4_sb[:], blk4[:])
            mask4_sb = sing.tile([4, P], bf16)
            nc.sync.dma_start(mask4_sb[:], mask4[:])
            tabs_sb = {}

            def load_tab(tn):
                c_t, s_t = tabs_in[tn]
                cs = sing.tile([P, 2, TOK], bf16, name=tn + "_c")
                nc.sync.dma_start(cs[:], c_t[:])
                ss = sing.tile([P, 2, TOK], bf16, name=tn + "_s")
                nc.sync.dma_start(ss[:], s_t[:])
                tabs_sb[tn] = (cs, ss)

            eps_sb = sing.tile([4, 1], f32)
            nc.vector.memset(eps_sb[:], float(EPS))
            bm3_sb = sing.tile([P, 1], f32)
            nc.vector.memset(bm3_sb[:], -3.0)
            bln64_sb = sing.tile([P, 1], f32)
            nc.vector.memset(bln64_sb[:], -LN64)
            resid = sing.tile([P, KO, TOK], f32)
            nc.sync.dma_start(resid[:], tgtT[:])
            load_tab("ckca")
            ones_c_sb = sing.tile([P, 1], bf16)
            ones_r128_sb = sing.tile([1, P], bf16)

            yT = sing.tile([P, KO, TOK], fp8, name="yT")
            yF = sing.tile([P, KO, TOK], bf16, name="yF")
            q4 = sing.tile([P, HG, 2, TOK], fp8, name="q4")
            xT = sing.tile([P, HH, TOK], fp8, name="xT")
            k_mine = sing.tile([P, HG, 2, TOK], fp8, name="k_mine")
            v_mine = sing.tile([P, 4, H, VW], fp8, name="v_mine")
            nc.vector.memset(v_mine[:, :, :, D : D + 1], 1.0)
            k_full = big.tile([P, HG, 2, SRCN], fp8, tag="k_full", name="k_full")
            v_full = big.tile([P, SKC, H, VW], fp8, tag="v_full", name="v_full")
            nc.vector.memset(v_full[:, :, :, D : D + 1], 1.0)

            def proj_quad(pq, wq, ysrc):
                """8 DR matmuls: quad projection into pq [128, 2, T]."""
                for j in range(2):
                    for kp in range(KOP):
                        nc.tensor.matmul(
                            pq[:, j, :],
                            wq[:, j, kp],
                            ysrc[:, 2 * kp : 2 * kp + 2, :],
                            start=(kp == 0),
                            stop=(kp == KOP - 1),
                            perf_mode=DR,
                        )

            def norm_rope_quad(pq, cos2_sb, sinpm_sb, dst):
                """pq PSUM [128(quad), 2, T] f32 at 8x scale -> dst fp8:
                rms-normed, qn-scaled (via tables), roped (j-swap)."""
                raw = work.tile([P, 2, TOK], bf16, tag="raw", name="raw")
                nc.scalar.copy(raw[:], pq[:])
                sq = work.tile([P, 2, TOK], bf16, tag="sq", name="sq")
                nc.vector.tensor_mul(sq[:], raw[:], raw[:])
                nb = big_ps("nb")  # ssq in bank 0, bc broadcast in bank 1
                ssq = nb[0:4, 0, :]
                for j in range(2):
                    nc.tensor.matmul(
                        ssq,
                        blk4_sb[:],
                        sq[:, j, :],
                        start=(j == 0),
                        stop=(j == 1),
                    )
                # rsqrt(mean+eps) = exp(-0.5*ln(mean+eps)); 1/(64*D) unscales
                # the x8 weight prescale (squared).
                lnt = stat.tile([4, TOK], bf16, tag="lnt", name="lnt")
                nc.scalar.activation(
                    lnt[:], ssq, AF.Ln, bias=eps_sb[:], scale=1.0 / (64 * D)
                )
                rs = stat.tile([4, TOK], bf16, tag="rs", name="rs")
                nc.scalar.activation(rs[:], lnt[:], AF.Exp, scale=-0.5)
                bc = nb[:, 1, :]
                nc.tensor.matmul(bc, mask4_sb[:], rs[:], start=True, stop=True)
                v1 = work.tile([P, 2, TOK], bf16, tag="v1", name="v1")
                for j in range(2):
                    nc.vector.tensor_mul(v1[:, j, :], raw[:, j, :], bc)
                t1 = work.tile([P, 2, TOK], bf16, tag="t1", name="t1")
                nc.vector.tensor_mul(t1[:], v1[:], cos2_sb[:])
                d0 = work.tile([P, 2, TOK], bf16, tag="d0", name="d0")
                for j in range(2):
                    nc.vector.tensor_mul(
                        d0[:, j, :], v1[:, 1 - j, :], sinpm_sb[:, j, :]
                    )
                nc.vector.tensor_add(dst, t1[:], d0[:])

            def kv_group_and_ag(
                g, ysrc, Wk_t, Wv_t, tabname, kv_in, kv_out, split=False
            ):
                """k (quad g) + v (4 heads) from my 512 rows -> AG.
                ysrc=None streams the core's src shard from DRAM.
                split=True gathers k first in its own collective so scores
                can start before v is on the wire."""
                if ysrc is None:
                    ysrc = work.tile(
                        [P, KO, TOK], fp8, tag="srcC", bufs=2, name="srcS"
                    )
                    nc.sync.dma_start(ysrc[:], srcTm[:])
                wk = wpool.tile([P, 2, KOP, 2, P], fp8, tag="wk2", name="wk")
                nc.sync.dma_start(
                    wk[:],
                    Wk_t[2 * g : 2 * g + 2].rearrange("h p a b m -> p h a b m"),
                )
                wv = wpool.tile([P, KOP, 2, 256], fp8, tag="wv", name="wv")
                nc.sync.dma_start(
                    wv[:], Wv_t[:, :, :, g * 256 : (g + 1) * 256]
                )

                def do_k():
                    pq = big_ps("pqk")
                    proj_quad(pq, wk, ysrc)
                    cs, ss = tabs_sb[tabname]
                    norm_rope_quad(pq, cs[:], ss[:], k_mine[:, g])
                    nc.sync.dma_start(
                        kv_in[:KSZ].rearrange(
                            "(p j t) -> p j t", p=P, j=2, t=TOK
                        ),
                        k_mine[:, g],
                    )

                def do_v():
                    for t in range(4):
                        pvt = small_ps("pv")
                        pv = pvt[:, 0, 0:256]
                        for kp in range(KOP):
                            nc.tensor.matmul(
                                pv,
                                ysrc[:, 2 * kp : 2 * kp + 2, t * P : (t + 1) * P],
                                wv[:, kp],
                                start=(kp == 0),
                                stop=(kp == KOP - 1),
                                perf_mode=DR,
                            )
                        nc.vector.tensor_copy(
                            v_mine[:, t, 4 * g : 4 * g + 4, 0:D],
                            pv.rearrange("p (h d) -> p h d", d=D),
                        )
                    nc.sync.dma_start(
                        kv_in[KSZ:].rearrange(
                            "(p a b c) -> p a b c", p=P, a=4, b=4, c=VW
                        ),
                        v_mine[:, :, 4 * g : 4 * g + 4, :],
                    )

                def ag(in_ap, out_ap):
                    nc.gpsimd.collective_compute(
                        "AllGather",
                        mybir.AluOpType.bypass,
                        replica_groups=groups,
                        ins=[in_ap],
                        outs=[out_ap],
                    )

                if split:
                    do_k()
                    ag(kv_in[:KSZ].opt(), kv_out[:, :KSZ].opt())
                    do_v()
                    ag(kv_in[KSZ:].opt(), kv_out[:, KSZ:].opt())
                else:
                    do_v()
                    do_k()
                    ag(kv_in.opt(), kv_out.opt())

            def ca_rep_group(g, Wk_t, Wv_t):
                """Replicated CA kv for quad g: every core computes k/v for
                all SRCN src tokens locally (no AllGather -> no CC-init
                latency before the first CA attention group)."""
                wk = wpool.tile([P, 2, KOP, 2, P], fp8, tag="wk2", name="wkr")
                nc.sync.dma_start(
                    wk[:],
                    Wk_t[2 * g : 2 * g + 2].rearrange("h p a b m -> p h a b m"),
                )
                wv = wpool.tile([P, KOP, 2, 256], fp8, tag="wv", name="wvr")
                nc.sync.dma_start(
                    wv[:], Wv_t[:, :, :, g * 256 : (g + 1) * 256]
                )
                for c4 in range(4):
                    srcC = work.tile(
                        [P, KO, TOK], fp8, tag="srcC", bufs=2, name="srcC"
                    )
                    nc.sync.dma_start(
                        srcC[:], srcT[:, :, c4 * TOK : (c4 + 1) * TOK]
                    )
                    ckc = work.tile(
                        [P, 2, TOK], bf16, tag="ckc", bufs=1, name="ckc"
                    )
                    nc.sync.dma_start(
                        ckc[:], ckf_c[:, :, c4 * TOK : (c4 + 1) * TOK]
                    )
                    cks = work.tile(
                        [P, 2, TOK], bf16, tag="cks", bufs=1, name="cks"
                    )
                    nc.sync.dma_start(
                        cks[:], ckf_s[:, :, c4 * TOK : (c4 + 1) * TOK]
                    )
                    pq = big_ps("pqk")
                    proj_quad(pq, wk, srcC)
                    norm_rope_quad(
                        pq, ckc[:], cks[:],
                        k_full[:, g, :, c4 * TOK : (c4 + 1) * TOK],
                    )
                    # v for this chunk right away (cheap; frees srcC slot)
                    for t in range(4):
                        pvt = small_ps("pv")
                        pv = pvt[:, 0, 0:256]
                        for kp in range(KOP):
                            nc.tensor.matmul(
                                pv,
                                srcC[:, 2 * kp : 2 * kp + 2, t * P : (t + 1) * P],
                                wv[:, kp],
                                start=(kp == 0),
                                stop=(kp == KOP - 1),
                                perf_mode=DR,
                            )
                        nc.vector.tensor_copy(
                            v_full[:, 4 * c4 + t, 4 * g : 4 * g + 4, 0:D],
                            pv.rearrange("p (h d) -> p h d", d=D),
                        )

            def scatter_group(g, kv_out):
                for r in range(NR):
                    nc.sync.dma_start(
                        k_full[:, g, :, r * TOK : (r + 1) * TOK],
                        kv_out[r, :KSZ].rearrange(
                            "(p j t) -> p j t", p=P, j=2, t=TOK
                        ),
                    )
                    nc.sync.dma_start(
                        v_full[:, r * 4 : (r + 1) * 4, 4 * g : 4 * g + 4, :],
                        kv_out[r, KSZ:].rearrange(
                            "(p a b c) -> p a b c", p=P, a=4, b=4, c=VW
                        ),
                    )

            def proj_q(Wt, tabname, quads=range(HG)):
                """y -> q (given quads), normed+roped into q4."""
                for g in quads:
                    wq = wpool.tile([P, 2, KOP, 2, P], fp8, tag="wk2", name="wq")
                    nc.sync.dma_start(
                        wq[:],
                        Wt[2 * g : 2 * g + 2].rearrange("h p a b m -> p h a b m"),
                    )
                    pq = big_ps("pq")
                    proj_quad(pq, wq, yT)
                    cs, ss = tabs_sb[tabname]
                    norm_rope_quad(pq, cs[:], ss[:], q4[:, g])

            def attention_group(hg, kdb):
                """scores+softmax+AV for quad hg (pairs j=0,1); fills
                xT[:, 2hg:2hg+2] with x_norm/8 (fp8)."""
                xraw = stat.tile(
                    [P, 2, TOK], bf16, tag="xraw", bufs=1, name="xraw"
                )
                dens4 = work.tile(
                    [P, 4, TOK], bf16, tag="dens", bufs=1, name="dens4"
                )
                for j in range(2):
                    px = psum.tile(
                        [VW, 2, TOK], f32, tag="px", bufs=1, name="px"
                    )
                    for kc in range(SKC):
                        ps = big_ps("ps")
                        for i in range(2):
                            b = 2 * j + i
                            nc.tensor.matmul(
                                ps[:, i, :],
                                k_full[
                                    32 * b : 32 * b + 32,
                                    hg,
                                    :,
                                    kc * P : (kc + 1) * P,
                                ],
                                q4[32 * b : 32 * b + 32, hg],
                                start=True,
                                stop=True,
                                perf_mode=DR,
                                tile_position=(32 * b, 0),
                            )
                        if kc % 2 == 0:
                            prob = probp.tile(
                                [P, 2, 2, TOK], fp8, tag="prob", name="prob"
                            )
                        if kc in DVE_EXP_KC:
                            nc.vector.tensor_scalar(
                                prob[:, kc % 2].bitcast(u8), ps[:],
                                EXP_S1, EXP_S2, MUL, ADD,
                            )
                        else:
                            nc.scalar.activation(
                                prob[:, kc % 2],
                                ps[:],
                                AF.Exp,
                                scale=1.0 / math.sqrt(D),
                                bias=bm3_sb[:],
                            )
                        if kc % 2 == 1:
                            j2 = kc - 1
                            for i in range(2):
                                nc.tensor.matmul(
                                    px[:, i, :],
                                    v_full[:, j2 : j2 + 2, hg * 4 + 2 * j + i, :],
                                    prob[:, :, i, :],
                                    start=(kc == 1),
                                    stop=(kc == SKC - 1),
                                    perf_mode=DR,
                                )
                    for i in range(2):
                        # denom row rides on partition 64 (ones column of v)
                        nc.vector.tensor_copy(
                            dens4[D : D + 1, 2 * j + i], px[D : D + 1, i, :]
                        )
                        nc.vector.tensor_copy(
                            xraw[i * D : (i + 1) * D, j], px[0:D, i, :]
                        )
                # reciprocals: 1/(64*den); the 64 unscales v and Wo x8 each,
                # making xT = x_norm/8 which Wo's x8 restores
                nc.sync.dma_start(
                    kdb[: 4 * TOK].rearrange("(o f t) -> o f t", o=1, f=4),
                    dens4[D : D + 1],
                )
                d4 = stat.tile([4, TOK], bf16, tag="d4", bufs=1, name="d4")
                nc.sync.dma_start(
                    d4[:], kdb[: 4 * TOK].rearrange("(f t) -> f t", f=4)
                )
                nc.scalar.activation(d4[:], d4[:], AF.Ln)
                rec4 = stat.tile([4, TOK], bf16, tag="rec4", bufs=1, name="rec4")
                nc.scalar.activation(
                    rec4[:], d4[:], AF.Exp, scale=-1.0, bias=bln64_sb[:4]
                )
                nc.sync.dma_start(
                    kdb[4 * TOK :].rearrange("(f t) -> f t", f=4), rec4[:]
                )
                rec_bc = work.tile(
                    [P, 2, TOK], bf16, tag="recbc", bufs=1, name="rec_bc"
                )
                for i in range(2):
                    src = bass.AP(
                        tensor=kdb.tensor,
                        offset=kdb.offset + 4 * TOK + i * TOK,
                        ap=[[0, D], [2 * TOK, 2], [1, TOK]],
                    )
                    nc.sync.dma_start(rec_bc[i * D : (i + 1) * D], src)
                for j in range(2):
                    nc.gpsimd.tensor_mul(
                        xT[:, 2 * hg + j], xraw[:, j], rec_bc[:, j]
                    )

            def wo_group(hg, Wo_t):
                """Wo partial for head-quad hg, accumulated into resid."""
                wo = wpool.tile([P, KO, 2, P], fp8, tag="wo", name="wo")
                nc.sync.dma_start(
                    wo[:], Wo_t[:, :, hg].rearrange("o p b m -> p o b m")
                )
                for oc in range(KO):
                    pot = small_ps("po")
                    po = pot[:, 0, :]
                    nc.tensor.matmul(
                        po,
                        wo[:, oc],
                        xT[:, 2 * hg : 2 * hg + 2, :],
                        start=True,
                        stop=True,
                        perf_mode=DR,
                    )
                    nc.vector.tensor_add(resid[:, oc], resid[:, oc], po)

            def rmsnorm_feat(dst):
                """resid f32 -> dst (fp8 or bf16): resid * rsqrt(mean sq)."""
                nb = big_ps("ynb")  # ssq in bank 0, bc broadcast in bank 1
                ssq = nb[0:1, 0, :]
                for c in range(KO):
                    sq = work.tile([P, TOK], bf16, tag="ysq", name="ynsq")
                    nc.vector.tensor_mul(sq[:], resid[:, c], resid[:, c])
                    nc.tensor.matmul(
                        ssq,
                        ones_c_sb[:],
                        sq[:],
                        start=(c == 0),
                        stop=(c == KO - 1),
                    )
                lnt = stat.tile([1, TOK], bf16, tag="lnt", name="ylnt")
                nc.scalar.activation(
                    lnt[:], ssq, AF.Ln, bias=eps_sb[:1], scale=1.0 / DIM
                )
                rs = stat.tile([1, TOK], bf16, tag="rs", name="yrs")
                nc.scalar.activation(rs[:], lnt[:], AF.Exp, scale=-0.5)
                bc = nb[:, 1, :]
                nc.tensor.matmul(bc, ones_r128_sb[:], rs[:], start=True, stop=True)
                for c in range(KO):
                    nc.vector.tensor_mul(dst[:, c], resid[:, c], bc)

            # ================= cross-attention =================
            # quad 0's kv is computed locally on every core (hides the CC
            # init latency); quads 1-3 go through pipelined AllGathers.
            kvi_ca = [
                dram.tile([KSZ + VSZ], fp8, tag=f"kvica{g}", name=f"kvica{g}")
                for g in range(1, HG)
            ]
            kvo_ca = [
                dram.tile([NR, KSZ + VSZ], fp8, tag=f"kvoca{g}", name=f"kvoca{g}")
                for g in range(1, HG)
            ]
            kdbs = [
                dram.tile([8 * TOK], bf16, tag=f"kdb{g}", name=f"kdb{g}")
                for g in range(HG)
            ]
            nc.sync.dma_start(ones_c_sb[:], ones_c[:])
            nc.sync.dma_start(ones_r128_sb[:], ones_r128[:])
            load_tab("cqca")
            rmsnorm_feat(yT)
            proj_q(caWq, "cqca")
            ca_rep_group(0, caWk, caWv)
            kv_group_and_ag(1, None, caWk, caWv, "ckca", kvi_ca[0], kvo_ca[0])
            kv_group_and_ag(2, None, caWk, caWv, "ckca", kvi_ca[1], kvo_ca[1])
            kv_group_and_ag(3, None, caWk, caWv, "ckca", kvi_ca[2], kvo_ca[2])
            load_tab("cqsa")
            load_tab("cksa")
            attention_group(0, kdbs[0])
            scatter_group(1, kvo_ca[0])
            attention_group(1, kdbs[1])
            wo_group(0, caWo)
            scatter_group(2, kvo_ca[1])
            attention_group(2, kdbs[2])
            wo_group(1, caWo)
            scatter_group(3, kvo_ca[2])
            wo_group(2, caWo)
            attention_group(3, kdbs[3])
            wo_group(3, caWo)

            # ================= self-attention =================
            rmsnorm_feat(yT)
            kvi_sa = [
                dram.tile([KSZ + VSZ], fp8, tag=f"kvisa{g}", name=f"kvisa{g}")
                for g in range(HG)
            ]
            kvo_sa = [
                dram.tile([NR, KSZ + VSZ], fp8, tag=f"kvosa{g}", name=f"kvosa{g}")
                for g in range(HG)
            ]
            kdbs2 = [
                dram.tile([8 * TOK], bf16, tag=f"kdc{g}", name=f"kdc{g}")
                for g in range(HG)
            ]
            kv_group_and_ag(0, yT, saWk, saWv, "cksa", kvi_sa[0], kvo_sa[0])
            kv_group_and_ag(1, yT, saWk, saWv, "cksa", kvi_sa[1], kvo_sa[1])
            proj_q(saWq, "cqsa")
            kv_group_and_ag(2, yT, saWk, saWv, "cksa", kvi_sa[2], kvo_sa[2])
            kv_group_and_ag(3, yT, saWk, saWv, "cksa", kvi_sa[3], kvo_sa[3])
            scatter_group(0, kvo_sa[0])
            attention_group(0, kdbs2[0])
            scatter_group(1, kvo_sa[1])
            attention_group(1, kdbs2[1])
            wo_group(0, saWo)
            scatter_group(2, kvo_sa[2])
            attention_group(2, kdbs2[2])
            wo_group(1, saWo)
            scatter_group(3, kvo_sa[3])
            wo_group(2, saWo)
            attention_group(3, kdbs2[3])
            wo_group(3, saWo)

            # ================= FFN (bf16) =================
            rmsnorm_feat(yF)
            for qtr in range(4):
                hT = htp.tile([P, 8, TOK], bf16, tag="hT", name="hT")
                for e in range(4):
                    w1 = w13p.tile([P, 2, KO, P], bf16, tag="w1", name="w1")
                    nc.sync.dma_start(
                        w1[:],
                        W1i[qtr * 8 + e * 2 : qtr * 8 + e * 2 + 2].rearrange(
                            "h p a m -> p h a m"
                        ),
                    )
                    w3 = w13p.tile([P, 2, KO, P], bf16, tag="w3", name="w3")
                    nc.sync.dma_start(
                        w3[:],
                        W3i[qtr * 8 + e * 2 : qtr * 8 + e * 2 + 2].rearrange(
                            "h p a m -> p h a m"
                        ),
                    )
                    for gg in range(2):
                        p13 = big_ps("p13")
                        p1 = p13[:, 0, :]
                        p3 = p13[:, 1, :]
                        for c in range(KO):
                            nc.tensor.matmul(
                                p1, w1[:, gg, c], yF[:, c],
                                start=(c == 0), stop=(c == KO - 1),
                            )
                        for c in range(KO):
                            nc.tensor.matmul(
                                p3, w3[:, gg, c], yF[:, c],
                                start=(c == 0), stop=(c == KO - 1),
                            )
                        s1 = stat.tile([P, TOK], bf16, tag="s1", name="s1")
                        nc.scalar.activation(s1[:], p1, AF.Silu)
                        nc.vector.tensor_mul(hT[:, e * 2 + gg], s1[:], p3)
                w2 = w2p.tile([P, KO, 8, P], bf16, tag="w2", name="w2")
                nc.sync.dma_start(
                    w2[:],
                    W2i[:, :, qtr * 8 : (qtr + 1) * 8].rearrange(
                        "o p a m -> p o a m"
                    ),
                )
                for oc in range(KO):
                    pot = small_ps("po2")
                    po = pot[:, 0, :]
                    for gg in range(8):
                        nc.tensor.matmul(
                            po, w2[:, oc, gg], hT[:, gg],
                            start=(gg == 0), stop=(gg == 7),
                        )
                    nc.vector.tensor_add(resid[:, oc], resid[:, oc], po)
                    if qtr == 3:
                        nc.sync.dma_start(outT[:, oc], resid[:, oc])

    _split_multiwait(nc)
    return nc


def _prep_inputs(inputs):
    """Full problem inputs -> list of 8 per-core in_maps."""
    tgt = np.asarray(inputs["tgt"], np.float32)
    src = np.asarray(inputs["src"], np.float32)
    tgt_pos = np.asarray(inputs["tgt_pos"], np.int32)
    src_pos = np.asarray(inputs["src_pos"], np.int32)

    pre_ca_w = np.asarray(inputs["pre_ca_w"], np.float32)
    pre_sa_w = np.asarray(inputs["pre_sa_w"], np.float32)
    pre_ffn_w = np.asarray(inputs["pre_ffn_w"], np.float32)

    def fold(Wname, w):
        return np.asarray(inputs[Wname], np.float32) * w[:, None]

    ca_Wq = fold("ca_Wq", pre_ca_w)
    ca_Wkv = np.asarray(inputs["ca_Wkv"], np.float32)
    ca_Wk, ca_Wv = ca_Wkv[:, :DIM], ca_Wkv[:, DIM:]
    ca_Wo = np.asarray(inputs["ca_Wo"], np.float32)
    sa_Wq = fold("sa_Wq", pre_sa_w)
    sa_Wkv = fold("sa_Wkv", pre_sa_w)
    sa_Wk, sa_Wv = sa_Wkv[:, :DIM], sa_Wkv[:, DIM:]
    sa_Wo = np.asarray(inputs["sa_Wo"], np.float32)
    W1 = fold("W1", pre_ffn_w)
    W3 = fold("W3", pre_ffn_w)
    W2 = np.asarray(inputs["W2"], np.float32)

    shared = {
        "caWq": _lhsT_dr(ca_Wq[:, _QPERM]),
        "caWk": _lhsT_dr(ca_Wk[:, _QPERM]),
        "caWv": _rhs_dr(ca_Wv),
        "caWo": _lhsT_dr(ca_Wo),
        "saWq": _lhsT_dr(sa_Wq[:, _QPERM]),
        "saWk": _lhsT_dr(sa_Wk[:, _QPERM]),
        "saWv": _rhs_dr(sa_Wv),
        "saWo": _lhsT_dr(sa_Wo),
        "W1": _lhsT_bf(W1),
        "W3": _lhsT_bf(W3),
        "W2": _lhsT_bf(W2),
    }

    blk4 = np.zeros((P, 4), np.float32)
    for m in range(4):
        blk4[32 * m : 32 * m + 32, m] = 1
    shared["blk4"] = blk4.astype(BF).copy()
    shared["mask4"] = blk4.T.astype(BF).copy()
    shared["ones_c"] = np.ones((P, 1), BF)
    shared["ones_r128"] = np.ones((1, P), BF)

    ca_qn = np.asarray(inputs["ca_qn"], np.float32)
    ca_kn = np.asarray(inputs["ca_kn"], np.float32)
    sa_qn = np.asarray(inputs["sa_qn"], np.float32)
    sa_kn = np.asarray(inputs["sa_kn"], np.float32)

    srcT_full = [_featmajor(src[s], F8) for s in range(B)]
    ckf = [_rope_tables_quad(src_pos[s], ca_kn) for s in range(B)]

    in_maps = []
    for c in range(NCORES):
        s, part = c // NR, c % NR
        rows = slice(part * TOK, (part + 1) * TOK)
        m = dict(shared)
        m["tgtT"] = _featmajor(tgt[s, rows], np.float32)
        m["srcT"] = srcT_full[s]
        m["srcTm"] = _featmajor(src[s, rows], F8)
        m["ckf_c"], m["ckf_s"] = ckf[s]
        tpos = tgt_pos[s, rows]
        spos = src_pos[s, rows]
        for tn, (pos, nv) in {
            "cqca": (tpos, ca_qn),
            "ckca": (spos, ca_kn),
            "cqsa": (tpos, sa_qn),
            "cksa": (tpos, sa_kn),
        }.items():
            ct, st = _rope_tables_quad(pos, nv)
            m[tn + "_c"] = ct
            m[tn + "_s"] = st
        in_maps.append(m)
    return in_maps


def _get_nc():
    if "nc" not in _cache:
        _cache["nc"] = _build_bass()
    return _cache["nc"]


def run(inputs, trace=False):
    """Run on 8 cores; returns (full_output, exec_time_ns_or_None)."""
    if trace:
        _install_ntff_hook()
    from concourse.bass_utils import run_bass_kernel_spmd

    in_maps = _prep_inputs(inputs)
    nc = _get_nc()
    res = run_bass_kernel_spmd(
        nc, in_maps, core_ids=list(range(NCORES)), trace=trace
    )
    out = np.empty((B, N, DIM), np.float32)
    for c in range(NCORES):
        s, part = c // NR, c % NR
        arr = np.asarray(res.results[c]["outT"])  # [128, 8, TOK]
        rows = slice(part * TOK, (part + 1) * TOK)
        out[s, rows] = np.transpose(arr, (2, 1, 0)).reshape(TOK, DIM)
    return out, res.exec_time_ns


def kernel(**inputs):
    out, _ = run(inputs, trace=False)
    return out


# revision 54
# speedup vs baseline: 1.0138x; 1.0138x over previous
"""Trainium2 Bass kernel for nn_CrossLayer (dense transformer layer), v5.

Sharding: sequence-parallel over 8 cores (2 samples x 4 token-chunks of 512).
Each core computes its 512 token rows through CA -> SA -> FFN.

- quad layout for q/k: head-quad tiles [128, 2, T] with partition
  p -> head 4g+(p//32), dim d = 32*j + p%32.  Rope's rotate-half becomes
  a free-dim (j) swap (no PE rotation matmul, no ACT rot copy); the cos/sin
  tables carry the qn/8 per-channel factors (host-folded).  rms-norm
  Ln/Exp batched per quad ([4,T] ACT ops).
- scores are DoubleRow matmuls on [32, 2, .] slices (explicit
  tile_position for the base-96 head); AV keeps the ones-column denominator
  trick.  Attention is fp8 e4m3 end to end; weights pre-scaled x8; /8s
  fold into the tables and the softmax-denominator exp bias (-ln 64).
- PSUM tags: "big" [128,2,512] bufs=3 (6 banks; projections, scores, FFN
  p1/p3 pairs) + "px" (2 banks, AV accumulate).  Scores triple-buffer
  against the softmax EXP on ACT so PE and ACT overlap.
- CA quad-0 k/v is computed replicated (every core does all 2048 src
  tokens) because the collective cores take ~60us to initialize after NEFF
  start; quads 1-3 AllGather in pipelined chunks that land during earlier
  groups' attention.  SA gathers all four quads (CC warm by then).
- FFN: bf16 (fp8 gives no matmul throughput on this HW and costs 1.1e-2+
  rel err); streams W1/W3 in 2-chunk tiles, hT double-buffered; the last
  quarter's residual chunks stream out as they finish.
- xT = xraw * 1/denom multiplies run on the otherwise-idle GpSimd engine.
"""

import math
import sys
import types

import numpy as np
import ml_dtypes

B, N, DIM, HID, H, D = 2, 2048, 1024, 4096, 16, 64
TOK = 512  # tokens per core
NCORES = 8
EPS = 1e-6
THETA = 10000.0
P = 128
KO = DIM // P  # 8 contraction chunks
KOP = KO // 2  # 4 DoubleRow pair-chunks
HH = H // 2  # 8 head pairs
HC = HID // P  # 32 hidden chunks
NR = 4  # ranks per replica group
SRCN = 2048  # gathered kv tokens
SKC = SRCN // P  # 16 key chunks of 128 tokens
VW = D + 1  # v columns + ones column
HG = 4  # head groups (quads: 4 heads each)
WS = 8.0  # attention weight pre-scale for fp8
LN64 = math.log(64.0)

BF = ml_dtypes.bfloat16
F8 = ml_dtypes.float8_e4m3

_cache = {}


def _quad_perm():
    """New output-channel order o' = 256*g + 128*j + p for quad layout:
    orig channel c = 64*(4g + p//32) + 32*j + (p%32)."""
    perm = np.empty(DIM, np.int64)
    for g in range(4):
        for j in range(2):
            for p in range(P):
                perm[256 * g + 128 * j + p] = (
                    64 * (4 * g + p // 32) + 32 * j + (p % 32)
                )
    return perm


_QPERM = _quad_perm()


def _lhsT_dr(W):
    """[K, M] -> [M//128, 128(K%128), K//256, 2, 128(M%128)] fp8 x8.
    Slice [mt][:, kp] is a DoubleRow lhsT [128, 2, 128]."""
    K, M = W.shape
    A = W.reshape(K // 256, 2, P, M // P, P)
    return (A.transpose(3, 2, 0, 1, 4) * WS).astype(F8).copy()


def _rhs_dr(W):
    """[K, M] -> [128, K//256, 2, M] fp8 x8 rhs-style DoubleRow moving."""
    K, M = W.shape
    A = W.reshape(K // 256, 2, P, M)
    return (A.transpose(2, 0, 1, 3) * WS).astype(F8).copy()


def _lhsT_bf(W):
    """[K, M] -> [M//128, 128(K%128), K//128, 128(M%128)] bf16 (unscaled)."""
    K, M = W.shape
    return W.reshape(K // P, P, M // P, P).transpose(2, 1, 0, 3).astype(BF).copy()


def _featmajor(x, dt):
    """[tok, dim] -> [128, dim//128, tok]."""
    n = x.shape[0]
    return x.T.reshape(DIM // P, P, n).transpose(1, 0, 2).astype(dt).copy()


def _rope_tables_quad(pos, nv):
    """pos [n] int32, nv [64] norm weights -> (cos2, sinpm) [128, 2, n] bf16.
    cos2[p,j,t] = cos(pos_t * invf[p%32]) * nv[32j + p%32] / 8
    sinpm[p,0,t] = -sin(.) * nv[32 + p%32] / 8   (d0[:,j] = v1[:,1-j]*sinpm[:,j])
    sinpm[p,1,t] = +sin(.) * nv[p%32] / 8
    """
    n = pos.shape[0]
    invf = 1.0 / (THETA ** (np.arange(0, D, 2, dtype=np.float64) / D))  # [32]
    pm32 = np.tile(np.arange(32), 4)  # p % 32 for p in 0..127
    ang = pos.astype(np.float64)[None, :] * invf[pm32][:, None]  # [128, n]
    c = np.cos(ang)
    s = np.sin(ang)
    nv = np.asarray(nv, np.float64)
    cos2 = np.empty((P, 2, n), np.float64)
    sinpm = np.empty((P, 2, n), np.float64)
    cos2[:, 0, :] = c * (nv[pm32] / WS)[:, None]
    cos2[:, 1, :] = c * (nv[32 + pm32] / WS)[:, None]
    sinpm[:, 0, :] = -s * (nv[32 + pm32] / WS)[:, None]
    sinpm[:, 1, :] = s * (nv[pm32] / WS)[:, None]
    return cos2.astype(BF).copy(), sinpm.astype(BF).copy()


def _install_ntff_hook():
    try:
        from trn_agent_boot.trn_boot import _ntff_profile_via_ctypes
    except ImportError:
        return
    if "antenv.axon_hooks" in sys.modules:
        return
    try:
        hook = _ntff_profile_via_ctypes("/opt/axon/libaxon_pjrt.so")
    except OSError:
        return
    mod = types.ModuleType("antenv.axon_hooks")
    mod.get_axon_ntff_profile_hook = lambda: hook
    mod.set_axon_ntff_profile_hook = lambda h: None
    sys.modules["antenv.axon_hooks"] = mod
    import antenv

    antenv.axon_hooks = mod


def _split_multiwait(nc):
    """This walrus only supports one sync-wait on CTRL-encoded instructions
    (Drain/NoOp); hoist excess waits onto single-wait NoOps placed before."""
    from concourse import mybir

    n_split = 0
    for f in nc.m.functions:
        for bb in f.blocks:
            new = []
            changed = False
            for ins in bb.instructions:
                si = ins.sync_info
                if (
                    si is not None
                    and si.on_wait is not None
                    and len(si.on_wait) > 1
                ):
                    waits = list(si.on_wait)
                    keep, rest = waits[:1], waits[1:]
                    for k, w in enumerate(rest):
                        new.append(
                            mybir.InstNoOp(
                                name=f"{ins.name}-wsplit{k}",
                                engine=ins.engine,
                                sync_info=mybir.SyncInfo(
                                    on_wait=[w], on_update=[]
                                ),
                                bass_nofuse=True,
                            )
                        )
                    si.on_wait = keep
                    n_split += 1
                    changed = True
                new.append(ins)
            if changed:
                bb.instructions = new
    return n_split


def _build_bass():
    from contextlib import ExitStack

    import concourse.bass as bass
    import concourse.tile as tile
    from concourse import mybir

    f32 = mybir.dt.float32
    bf16 = mybir.dt.bfloat16
    fp8 = mybir.dt.float8e4
    u8 = mybir.dt.uint8
    AF = mybir.ActivationFunctionType
    DR = mybir.MatmulPerfMode.DoubleRow
    MUL = mybir.AluOpType.mult
    ADD = mybir.AluOpType.add
    # Schraudolph-style exp straight to fp8 e4m3 bits:
    # bits = round(score * A8/8 + (56 - C8 - 3*A8)); uint8 convert saturates
    # negatives to 0.  Softmax-level error matches exact-exp + fp8 rounding.
    A8 = 8.0 / math.log(2.0)
    EXP_S1 = A8 / 8.0
    EXP_S2 = 56.0 - 0.8 - 3.0 * A8
    # DVE bit-trick exp offload disabled: a run with it produced a one-off
    # NaN (suspected dep-tracking race on the bitcast write); the ~20us gain
    # is within run noise and not worth the correctness risk.
    DVE_EXP_KC = frozenset()

    nc = bass.Bass(num_devices=NCORES)

    def inp(name, shape, dt=fp8):
        return nc.dram_tensor(name, shape, dt, kind="ExternalInput")

    tgtT = inp("tgtT", [P, KO, TOK], f32)
    srcT = inp("srcT", [P, KO, SRCN])  # full sample (CA quad-0 kv replicated)
    srcTm = inp("srcTm", [P, KO, TOK])  # this core's 512-row src shard
    # rope/norm tables: [P, 2, TOK] bf16 per (pos-set, norm-vec);
    # ckca_f covers all SRCN src positions (streamed, for replicated quad 0)
    tab_names = ["cqca", "ckca", "cqsa", "cksa"]
    tabs_in = {}
    for tn in tab_names:
        tabs_in[tn] = (
            inp(tn + "_c", [P, 2, TOK], bf16),
            inp(tn + "_s", [P, 2, TOK], bf16),
        )
    ckf_c = inp("ckf_c", [P, 2, SRCN], bf16)
    ckf_s = inp("ckf_s", [P, 2, SRCN], bf16)
    caWq = inp("caWq", [HH, P, KOP, 2, P])
    caWk = inp("caWk", [HH, P, KOP, 2, P])
    caWv = inp("caWv", [P, KOP, 2, DIM])
    caWo = inp("caWo", [KO, P, KOP, 2, P])
    saWq = inp("saWq", [HH, P, KOP, 2, P])
    saWk = inp("saWk", [HH, P, KOP, 2, P])
    saWv = inp("saWv", [P, KOP, 2, DIM])
    saWo = inp("saWo", [KO, P, KOP, 2, P])
    W1i = inp("W1", [HC, P, KO, P], bf16)
    W3i = inp("W3", [HC, P, KO, P], bf16)
    W2i = inp("W2", [KO, P, HC, P], bf16)
    blk4 = inp("blk4", [P, 4], bf16)  # per-head ssq lhsT (block ones)
    mask4 = inp("mask4", [4, P], bf16)  # rsqrt bcast lhsT (block ones)
    ones_c = inp("ones_c", [P, 1], bf16)  # y-norm ssq lhsT
    ones_r128 = inp("ones_r128", [1, P], bf16)  # y-norm bcast lhsT

    outT = nc.dram_tensor("outT", [P, KO, TOK], f32, kind="ExternalOutput")

    groups = [[0, 1, 2, 3], [4, 5, 6, 7]]
    KSZ = P * 2 * TOK  # k fp8 words per rank per head-group
    VSZ = P * 4 * 4 * VW  # v fp8 words per rank per head-group

    with tile.TileContext(nc) as tc:
        ctx = ExitStack()
        with ctx:
            sing = ctx.enter_context(tc.tile_pool(name="sing", bufs=1))
            big = ctx.enter_context(tc.tile_pool(name="big", bufs=1))
            wpool = ctx.enter_context(tc.tile_pool(name="wpool", bufs=2))
            w13p = ctx.enter_context(tc.tile_pool(name="w13p", bufs=2))
            w2p = ctx.enter_context(tc.tile_pool(name="w2p", bufs=1))
            htp = ctx.enter_context(tc.tile_pool(name="htp", bufs=2))
            work = ctx.enter_context(tc.tile_pool(name="work", bufs=2))
            probp = ctx.enter_context(tc.tile_pool(name="probp", bufs=2))
            stat = ctx.enter_context(tc.tile_pool(name="stat", bufs=2))
            dram = ctx.enter_context(
                tc.tile_pool(name="dram", bufs=1, space="DRAM")
            )
            # PSUM: "big" [128,2,512] bufs=3 (6 banks) + "px" (2 banks)
            psum = ctx.enter_context(tc.tile_pool(name="psum", bufs=3, space="PSUM"))

            def big_ps(name):
                return psum.tile([P, 2, TOK], f32, tag="big", name=name)

            def small_ps(name, part=P):
                t = psum.tile([part, 2, TOK], f32, tag="big", name=name)
                return t

            # ---- resident tiles (DMA order: CA-kv critical path first)
            blk4_sb = sing.tile([P, 4], bf16)
            nc.sync.dma_start(blk4_sb[:], blk4[:])
            mask4_sb = sing.tile([4, P], bf16)
            nc.sync.dma_start(mask4_sb[:], mask4[:])
            tabs_sb = {}

            def load_tab(tn):
                c_t, s_t = tabs_in[tn]
                cs = sing.tile([P, 2, TOK], bf16, name=tn + "_c")
                nc.sync.dma_start(cs[:], c_t[:])
                ss = sing.tile([P, 2, TOK], bf16, name=tn + "_s")
                nc.sync.dma_start(ss[:], s_t[:])
                tabs_sb[tn] = (cs, ss)

            eps_sb = sing.tile([4, 1], f32)
            nc.vector.memset(eps_sb[:], float(EPS))
            bm3_sb = sing.tile([P, 1], f32)
            nc.vector.memset(bm3_sb[:], -3.0)
            bln64_sb = sing.tile([P, 1], f32)
            nc.vector.memset(bln64_sb[:], -LN64)
            resid = sing.tile([P, KO, TOK], f32)
            nc.sync.dma_start(resid[:], tgtT[:])
            load_tab("ckca")
            ones_c_sb = sing.tile([P, 1], bf16)
            ones_r128_sb = sing.tile([1, P], bf16)

            yT = sing.tile([P, KO, TOK], fp8, name="yT")
            yF = sing.tile([P, KO, TOK], bf16, name="yF")
            q4 = sing.tile([P, HG, 2, TOK], fp8, name="q4")
            xT = sing.tile([P, HH, TOK], fp8, name="xT")
            k_mine = sing.tile([P, HG, 2, TOK], fp8, name="k_mine")
            v_mine = sing.tile([P, 4, H, VW], fp8, name="v_mine")
            nc.vector.memset(v_mine[:, :, :, D : D + 1], 1.0)
            k_full = big.tile([P, HG, 2, SRCN], fp8, tag="k_full", name="k_full")
            v_full = big.tile([P, SKC, H, VW], fp8, tag="v_full", name="v_full")
            nc.vector.memset(v_full[:, :, :, D : D + 1], 1.0)

            def proj_quad(pq, wq, ysrc):
                """8 DR matmuls: quad projection into pq [128, 2, T]."""
                for j in range(2):
                    for kp in range(KOP):
                        nc.tensor.matmul(
                            pq[:, j, :],
                            wq[:, j, kp],
                            ysrc[:, 2 * kp : 2 * kp + 2, :],
                            start=(kp == 0),
                            stop=(kp == KOP - 1),
                            perf_mode=DR,
                        )

            def norm_rope_quad(pq, cos2_sb, sinpm_sb, dst):
                """pq PSUM [128(quad), 2, T] f32 at 8x scale -> dst fp8:
                rms-normed, qn-scaled (via tables), roped (j-swap)."""
                raw = work.tile([P, 2, TOK], bf16, tag="raw", name="raw")
                nc.scalar.copy(raw[:], pq[:])
                sq = work.tile([P, 2, TOK], bf16, tag="sq", name="sq")
                nc.vector.tensor_mul(sq[:], raw[:], raw[:])
                nb = big_ps("nb")  # ssq in bank 0, bc broadcast in bank 1
                ssq = nb[0:4, 0, :]
                for j in range(2):
                    nc.tensor.matmul(
                        ssq,
                        blk4_sb[:],
                        sq[:, j, :],
                        start=(j == 0),
                        stop=(j == 1),
                    )
                # rsqrt(mean+eps) = exp(-0.5*ln(mean+eps)); 1/(64*D) unscales
                # the x8 weight prescale (squared).
                lnt = stat.tile([4, TOK], bf16, tag="lnt", name="lnt")
                nc.scalar.activation(
                    lnt[:], ssq, AF.Ln, bias=eps_sb[:], scale=1.0 / (64 * D)
                )
                rs = stat.tile([4, TOK], bf16, tag="rs", name="rs")
                nc.scalar.activation(rs[:], lnt[:], AF.Exp, scale=-0.5)
                bc = nb[:, 1, :]
                nc.tensor.matmul(bc, mask4_sb[:], rs[:], start=True, stop=True)
                v1 = work.tile([P, 2, TOK], bf16, tag="v1", name="v1")
                for j in range(2):
                    nc.vector.tensor_mul(v1[:, j, :], raw[:, j, :], bc)
                t1 = work.tile([P, 2, TOK], bf16, tag="t1", name="t1")
                nc.vector.tensor_mul(t1[:], v1[:], cos2_sb[:])
                d0 = work.tile([P, 2, TOK], bf16, tag="d0", name="d0")
                for j in range(2):
                    nc.vector.tensor_mul(
                        d0[:, j, :], v1[:, 1 - j, :], sinpm_sb[:, j, :]
                    )
                nc.vector.tensor_add(dst, t1[:], d0[:])

            def kv_group_and_ag(
                g, ysrc, Wk_t, Wv_t, tabname, kv_in, kv_out, split=False
            ):
                """k (quad g) + v (4 heads) from my 512 rows -> AG.
                ysrc=None streams the core's src shard from DRAM.
                split=True gathers k first in its own collective so scores
                can start before v is on the wire."""
                if ysrc is None:
                    ysrc = work.tile(
                        [P, KO, TOK], fp8, tag="srcC", bufs=2, name="srcS"
                    )
                    nc.sync.dma_start(ysrc[:], srcTm[:])
                wk = wpool.tile([P, 2, KOP, 2, P], fp8, tag="wk2", name="wk")
                nc.sync.dma_start(
                    wk[:],
                    Wk_t[2 * g : 2 * g + 2].rearrange("h p a b m -> p h a b m"),
                )
                wv = wpool.tile([P, KOP, 2, 256], fp8, tag="wv", name="wv")
                nc.sync.dma_start(
                    wv[:], Wv_t[:, :, :, g * 256 : (g + 1) * 256]
                )

                def do_k():
                    pq = big_ps("pqk")
                    proj_quad(pq, wk, ysrc)
                    cs, ss = tabs_sb[tabname]
                    norm_rope_quad(pq, cs[:], ss[:], k_mine[:, g])
                    nc.sync.dma_start(
                        kv_in[:KSZ].rearrange(
                            "(p j t) -> p j t", p=P, j=2, t=TOK
                        ),
                        k_mine[:, g],
                    )

                def do_v():
                    for t in range(4):
                        pvt = small_ps("pv")
                        pv = pvt[:, 0, 0:256]
                        for kp in range(KOP):
                            nc.tensor.matmul(
                                pv,
                                ysrc[:, 2 * kp : 2 * kp + 2, t * P : (t + 1) * P],
                                wv[:, kp],
                                start=(kp == 0),
                                stop=(kp == KOP - 1),
                                perf_mode=DR,
                            )
                        nc.vector.tensor_copy(
                            v_mine[:, t, 4 * g : 4 * g + 4, 0:D],
                            pv.rearrange("p (h d) -> p h d", d=D),
                        )
                    nc.sync.dma_start(
                        kv_in[KSZ:].rearrange(
                            "(p a b c) -> p a b c", p=P, a=4, b=4, c=VW
                        ),
                        v_mine[:, :, 4 * g : 4 * g + 4, :],
                    )

                def ag(in_ap, out_ap):
                    nc.gpsimd.collective_compute(
                        "AllGather",
                        mybir.AluOpType.bypass,
                        replica_groups=groups,
                        ins=[in_ap],
                        outs=[out_ap],
                    )

                if split:
                    do_k()
                    ag(kv_in[:KSZ].opt(), kv_out[:, :KSZ].opt())
                    do_v()
                    ag(kv_in[KSZ:].opt(), kv_out[:, KSZ:].opt())
                else:
                    do_v()
                    do_k()
                    ag(kv_in.opt(), kv_out.opt())

            def ca_rep_group(g, Wk_t, Wv_t):
                """Replicated CA kv for quad g: every core computes k/v for
                all SRCN src tokens locally (no AllGather -> no CC-init
                latency before the first CA attention group)."""
                wk = wpool.tile([P, 2, KOP, 2, P], fp8, tag="wk2", name="wkr")
                nc.sync.dma_start(
                    wk[:],
                    Wk_t[2 * g : 2 * g + 2].rearrange("h p a b m -> p h a b m"),
                )
                wv = wpool.tile([P, KOP, 2, 256], fp8, tag="wv", name="wvr")
                nc.sync.dma_start(
                    wv[:], Wv_t[:, :, :, g * 256 : (g + 1) * 256]
                )
                for c4 in range(4):
                    srcC = work.tile(
                        [P, KO, TOK], fp8, tag="srcC", bufs=2, name="srcC"
                    )
                    nc.sync.dma_start(
                        srcC[:], srcT[:, :, c4 * TOK : (c4 + 1) * TOK]
                    )
                    ckc = work.tile(
                        [P, 2, TOK], bf16, tag="ckc", bufs=1, name="ckc"
                    )
                    nc.sync.dma_start(
                        ckc[:], ckf_c[:, :, c4 * TOK : (c4 + 1) * TOK]
                    )
                    cks = work.tile(
                        [P, 2, TOK], bf16, tag="cks", bufs=1, name="cks"
                    )
                    nc.sync.dma_start(
                        cks[:], ckf_s[:, :, c4 * TOK : (c4 + 1) * TOK]
                    )
                    pq = big_ps("pqk")
                    proj_quad(pq, wk, srcC)
                    norm_rope_quad(
                        pq, ckc[:], cks[:],
                        k_full[:, g, :, c4 * TOK : (c4 + 1) * TOK],
                    )
                    # v for this chunk right away (cheap; frees srcC slot)
                    for t in range(4):
                        pvt = small_ps("pv")
                        pv = pvt[:, 0, 0:256]
                        for kp in range(KOP):
                            nc.tensor.matmul(
                                pv,
                                srcC[:, 2 * kp : 2 * kp + 2, t * P : (t + 1) * P],
                                wv[:, kp],
                                start=(kp == 0),
                                stop=(kp == KOP - 1),
                                perf_mode=DR,
                            )
                        nc.vector.tensor_copy(
                            v_full[:, 4 * c4 + t, 4 * g : 4 * g + 4, 0:D],
                            pv.rearrange("p (h d) -> p h d", d=D),
                        )

            def scatter_group(g, kv_out):
                for r in range(NR):
                    nc.sync.dma_start(
                        k_full[:, g, :, r * TOK : (r + 1) * TOK],
                        kv_out[r, :KSZ].rearrange(
                            "(p j t) -> p j t", p=P, j=2, t=TOK
                        ),
                    )
                    nc.sync.dma_start(
                        v_full[:, r * 4 : (r + 1) * 4, 4 * g : 4 * g + 4, :],
                        kv_out[r, KSZ:].rearrange(
                            "(p a b c) -> p a b c", p=P, a=4, b=4, c=VW
                        ),
                    )

            def proj_q(Wt, tabname, quads=range(HG)):
                """y -> q (given quads), normed+roped into q4."""
                for g in quads:
                    wq = wpool.tile([P, 2, KOP, 2, P], fp8, tag="wk2", name="wq")
                    nc.sync.dma_start(
                        wq[:],
                        Wt[2 * g : 2 * g + 2].rearrange("h p a b m -> p h a b m"),
                    )
                    pq = big_ps("pq")
                    proj_quad(pq, wq, yT)
                    cs, ss = tabs_sb[tabname]
                    norm_rope_quad(pq, cs[:], ss[:], q4[:, g])

            def attention_group(hg, kdb):
                """scores+softmax+AV for quad hg (pairs j=0,1); fills
                xT[:, 2hg:2hg+2] with x_norm/8 (fp8)."""
                xraw = stat.tile(
                    [P, 2, TOK], bf16, tag="xraw", bufs=1, name="xraw"
                )
                dens4 = work.tile(
                    [P, 4, TOK], bf16, tag="dens", bufs=1, name="dens4"
                )
                for j in range(2):
                    px = psum.tile(
                        [VW, 2, TOK], f32, tag="px", bufs=1, name="px"
                    )
                    for kc in range(SKC):
                        ps = big_ps("ps")
                        for i in range(2):
                            b = 2 * j + i
                            nc.tensor.matmul(
                                ps[:, i, :],
                                k_full[
                                    32 * b : 32 * b + 32,
                                    hg,
                                    :,
                                    kc * P : (kc + 1) * P,
                                ],
                                q4[32 * b : 32 * b + 32, hg],
                                start=True,
                                stop=True,
                                perf_mode=DR,
                                tile_position=(32 * b, 0),
                            )
                        if kc % 2 == 0:
                            prob = probp.tile(
                                [P, 2, 2, TOK], fp8, tag="prob", name="prob"
                            )
                        if kc in DVE_EXP_KC:
                            nc.vector.tensor_scalar(
                                prob[:, kc % 2].bitcast(u8), ps[:],
                                EXP_S1, EXP_S2, MUL, ADD,
                            )
                        else:
                            nc.scalar.activation(
                                prob[:, kc % 2],
                                ps[:],
                                AF.Exp,
                                scale=1.0 / math.sqrt(D),
                                bias=bm3_sb[:],
                            )
                        if kc % 2 == 1:
                            j2 = kc - 1
                            for i in range(2):
                                nc.tensor.matmul(
                                    px[:, i, :],
                                    v_full[:, j2 : j2 + 2, hg * 4 + 2 * j + i, :],
                                    prob[:, :, i, :],
                                    start=(kc == 1),
                                    stop=(kc == SKC - 1),
                                    perf_mode=DR,
                                )
                    for i in range(2):
                        # denom row rides on partition 64 (ones column of v)
                        nc.vector.tensor_copy(
                            dens4[D : D + 1, 2 * j + i], px[D : D + 1, i, :]
                        )
                        nc.vector.tensor_copy(
                            xraw[i * D : (i + 1) * D, j], px[0:D, i, :]
                        )
                # reciprocals: 1/(64*den); the 64 unscales v and Wo x8 each,
                # making xT = x_norm/8 which Wo's x8 restores
                nc.sync.dma_start(
                    kdb[: 4 * TOK].rearrange("(o f t) -> o f t", o=1, f=4),
                    dens4[D : D + 1],
                )
                d4 = stat.tile([4, TOK], bf16, tag="d4", bufs=1, name="d4")
                nc.sync.dma_start(
                    d4[:], kdb[: 4 * TOK].rearrange("(f t) -> f t", f=4)
                )
                nc.scalar.activation(d4[:], d4[:], AF.Ln)
                rec4 = stat.tile([4, TOK], bf16, tag="rec4", bufs=1, name="rec4")
                nc.scalar.activation(
                    rec4[:], d4[:], AF.Exp, scale=-1.0, bias=bln64_sb[:4]
                )
                nc.sync.dma_start(
                    kdb[4 * TOK :].rearrange("(f t) -> f t", f=4), rec4[:]
                )
                rec_bc = work.tile(
                    [P, 2, TOK], bf16, tag="recbc", bufs=1, name="rec_bc"
                )
                for i in range(2):
                    src = bass.AP(
                        tensor=kdb.tensor,
                        offset=kdb.offset + 4 * TOK + i * TOK,
                        ap=[[0, D], [2 * TOK, 2], [1, TOK]],
                    )
                    nc.sync.dma_start(rec_bc[i * D : (i + 1) * D], src)
                for j in range(2):
                    nc.gpsimd.tensor_mul(
                        xT[:, 2 * hg + j], xraw[:, j], rec_bc[:, j]
                    )

            def wo_group(hg, Wo_t):
                """Wo partial for head-quad hg, accumulated into resid."""
                wo = wpool.tile([P, KO, 2, P], fp8, tag="wo", name="wo")
                nc.sync.dma_start(
                    wo[:], Wo_t[:, :, hg].rearrange("o p b m -> p o b m")
                )
                for oc in range(KO):
                    pot = small_ps("po")
                    po = pot[:, 0, :]
                    nc.tensor.matmul(
                        po,
                        wo[:, oc],
                        xT[:, 2 * hg : 2 * hg + 2, :],
                        start=True,
                        stop=True,
                        perf_mode=DR,
                    )
                    nc.vector.tensor_add(resid[:, oc], resid[:, oc], po)

            def rmsnorm_feat(dst):
                """resid f32 -> dst (fp8 or bf16): resid * rsqrt(mean sq)."""
                nb = big_ps("ynb")  # ssq in bank 0, bc broadcast in bank 1
                ssq = nb[0:1, 0, :]
                for c in range(KO):
                    sq = work.tile([P, TOK], bf16, tag="ysq", name="ynsq")
                    nc.vector.tensor_mul(sq[:], resid[:, c], resid[:, c])
                    nc.tensor.matmul(
                        ssq,
                        ones_c_sb[:],
                        sq[:],
                        start=(c == 0),
                        stop=(c == KO - 1),
                    )
                lnt = stat.tile([1, TOK], bf16, tag="lnt", name="ylnt")
                nc.scalar.activation(
                    lnt[:], ssq, AF.Ln, bias=eps_sb[:1], scale=1.0 / DIM
                )
                rs = stat.tile([1, TOK], bf16, tag="rs", name="yrs")
                nc.scalar.activation(rs[:], lnt[:], AF.Exp, scale=-0.5)
                bc = nb[:, 1, :]
                nc.tensor.matmul(bc, ones_r128_sb[:], rs[:], start=True, stop=True)
                for c in range(KO):
                    nc.vector.tensor_mul(dst[:, c], resid[:, c], bc)

            # ================= cross-attention =================
            # quad 0's kv is computed locally on every core (hides the CC
            # init latency); quads 1-3 go through pipelined AllGathers.
            kvi_ca = [
                dram.tile([KSZ + VSZ], fp8, tag=f"kvica{g}", name=f"kvica{g}")
                for g in range(1, HG)
            ]
            kvo_ca = [
                dram.tile([NR, KSZ + VSZ], fp8, tag=f"kvoca{g}", name=f"kvoca{g}")
                for g in range(1, HG)
            ]
            kdbs = [
                dram.tile([8 * TOK], bf16, tag=f"kdb{g}", name=f"kdb{g}")
                for g in range(HG)
            ]
            nc.sync.dma_start(ones_c_sb[:], ones_c[:])
            nc.sync.dma_start(ones_r128_sb[:], ones_r128[:])
            load_tab("cqca")
            rmsnorm_feat(yT)
            proj_q(caWq, "cqca")
            ca_rep_group(0, caWk, caWv)
            kv_group_and_ag(1, None, caWk, caWv, "ckca", kvi_ca[0], kvo_ca[0])
            load_tab("cqsa")
            load_tab("cksa")
            attention_group(0, kdbs[0])
            kv_group_and_ag(2, None, caWk, caWv, "ckca", kvi_ca[1], kvo_ca[1])
            scatter_group(1, kvo_ca[0])
            attention_group(1, kdbs[1])
            wo_group(0, caWo)
            kv_group_and_ag(3, None, caWk, caWv, "ckca", kvi_ca[2], kvo_ca[2])
            scatter_group(2, kvo_ca[1])
            attention_group(2, kdbs[2])
            wo_group(1, caWo)
            scatter_group(3, kvo_ca[2])
            attention_group(3, kdbs[3])
            wo_group(2, caWo)
            wo_group(3, caWo)

            # ================= self-attention =================
            rmsnorm_feat(yT)
            kvi_sa = [
                dram.tile([KSZ + VSZ], fp8, tag=f"kvisa{g}", name=f"kvisa{g}")
                for g in range(HG)
            ]
            kvo_sa = [
                dram.tile([NR, KSZ + VSZ], fp8, tag=f"kvosa{g}", name=f"kvosa{g}")
                for g in range(HG)
            ]
            kdbs2 = [
                dram.tile([8 * TOK], bf16, tag=f"kdc{g}", name=f"kdc{g}")
                for g in range(HG)
            ]
            for g in range(HG):
                kv_group_and_ag(
                    g, yT, saWk, saWv, "cksa", kvi_sa[g], kvo_sa[g]
                )
                if g == 0:
                    proj_q(saWq, "cqsa")
            for hg in range(HG):
                scatter_group(hg, kvo_sa[hg])
                attention_group(hg, kdbs2[hg])
                if hg >= 1:
                    wo_group(hg - 1, saWo)
            wo_group(HG - 1, saWo)

            # ================= FFN (bf16) =================
            rmsnorm_feat(yF)
            for qtr in range(4):
                hT = htp.tile([P, 8, TOK], bf16, tag="hT", name="hT")
                for e in range(4):
                    w1 = w13p.tile([P, 2, KO, P], bf16, tag="w1", name="w1")
                    nc.sync.dma_start(
                        w1[:],
                        W1i[qtr * 8 + e * 2 : qtr * 8 + e * 2 + 2].rearrange(
                            "h p a m -> p h a m"
                        ),
                    )
                    w3 = w13p.tile([P, 2, KO, P], bf16, tag="w3", name="w3")
                    nc.sync.dma_start(
                        w3[:],
                        W3i[qtr * 8 + e * 2 : qtr * 8 + e * 2 + 2].rearrange(
                            "h p a m -> p h a m"
                        ),
                    )
                    for gg in range(2):
                        p13 = big_ps("p13")
                        p1 = p13[:, 0, :]
                        p3 = p13[:, 1, :]
                        for c in range(KO):
                            nc.tensor.matmul(
                                p1, w1[:, gg, c], yF[:, c],
                                start=(c == 0), stop=(c == KO - 1),
                            )
                        for c in range(KO):
                            nc.tensor.matmul(
                                p3, w3[:, gg, c], yF[:, c],
                                start=(c == 0), stop=(c == KO - 1),
                            )
                        s1 = stat.tile([P, TOK], bf16, tag="s1", name="s1")
                        nc.scalar.activation(s1[:], p1, AF.Silu)
                        nc.vector.tensor_mul(hT[:, e * 2 + gg], s1[:], p3)
                w2 = w2p.tile([P, KO, 8, P], bf16, tag="w2", name="w2")
                nc.sync.dma_start(
                    w2[:],
                    W2i[:, :, qtr * 8 : (qtr + 1) * 8].rearrange(
                        "o p a m -> p o a m"
                    ),
                )
                for oc in range(KO):
                    pot = small_ps("po2")
                    po = pot[:, 0, :]
                    for gg in range(8):
                        nc.tensor.matmul(
                            po, w2[:, oc, gg], hT[:, gg],
                            start=(gg == 0), stop=(gg == 7),
                        )
                    nc.vector.tensor_add(resid[:, oc], resid[:, oc], po)
                    if qtr == 3:
                        nc.sync.dma_start(outT[:, oc], resid[:, oc])

    _split_multiwait(nc)
    return nc


def _prep_inputs(inputs):
    """Full problem inputs -> list of 8 per-core in_maps."""
    tgt = np.asarray(inputs["tgt"], np.float32)
    src = np.asarray(inputs["src"], np.float32)
    tgt_pos = np.asarray(inputs["tgt_pos"], np.int32)
    src_pos = np.asarray(inputs["src_pos"], np.int32)

    pre_ca_w = np.asarray(inputs["pre_ca_w"], np.float32)
    pre_sa_w = np.asarray(inputs["pre_sa_w"], np.float32)
    pre_ffn_w = np.asarray(inputs["pre_ffn_w"], np.float32)

    def fold(Wname, w):
        return np.asarray(inputs[Wname], np.float32) * w[:, None]

    ca_Wq = fold("ca_Wq", pre_ca_w)
    ca_Wkv = np.asarray(inputs["ca_Wkv"], np.float32)
    ca_Wk, ca_Wv = ca_Wkv[:, :DIM], ca_Wkv[:, DIM:]
    ca_Wo = np.asarray(inputs["ca_Wo"], np.float32)
    sa_Wq = fold("sa_Wq", pre_sa_w)
    sa_Wkv = fold("sa_Wkv", pre_sa_w)
    sa_Wk, sa_Wv = sa_Wkv[:, :DIM], sa_Wkv[:, DIM:]
    sa_Wo = np.asarray(inputs["sa_Wo"], np.float32)
    W1 = fold("W1", pre_ffn_w)
    W3 = fold("W3", pre_ffn_w)
    W2 = np.asarray(inputs["W2"], np.float32)

    shared = {
        "caWq": _lhsT_dr(ca_Wq[:, _QPERM]),
        "caWk": _lhsT_dr(ca_Wk[:, _QPERM]),
        "caWv": _rhs_dr(ca_Wv),
        "caWo": _lhsT_dr(ca_Wo),
        "saWq": _lhsT_dr(sa_Wq[:, _QPERM]),
        "saWk": _lhsT_dr(sa_Wk[:, _QPERM]),
        "saWv": _rhs_dr(sa_Wv),
        "saWo": _lhsT_dr(sa_Wo),
        "W1": _lhsT_bf(W1),
        "W3": _lhsT_bf(W3),
        "W2": _lhsT_bf(W2),
    }

    blk4 = np.zeros((P, 4), np.float32)
    for m in range(4):
        blk4[32 * m : 32 * m + 32, m] = 1
    shared["blk4"] = blk4.astype(BF).copy()
    shared["mask4"] = blk4.T.astype(BF).copy()
    shared["ones_c"] = np.ones((P, 1), BF)
    shared["ones_r128"] = np.ones((1, P), BF)

    ca_qn = np.asarray(inputs["ca_qn"], np.float32)
    ca_kn = np.asarray(inputs["ca_kn"], np.float32)
    sa_qn = np.asarray(inputs["sa_qn"], np.float32)
    sa_kn = np.asarray(inputs["sa_kn"], np.float32)

    srcT_full = [_featmajor(src[s], F8) for s in range(B)]
    ckf = [_rope_tables_quad(src_pos[s], ca_kn) for s in range(B)]

    in_maps = []
    for c in range(NCORES):
        s, part = c // NR, c % NR
        rows = slice(part * TOK, (part + 1) * TOK)
        m = dict(shared)
        m["tgtT"] = _featmajor(tgt[s, rows], np.float32)
        m["srcT"] = srcT_full[s]
        m["srcTm"] = _featmajor(src[s, rows], F8)
        m["ckf_c"], m["ckf_s"] = ckf[s]
        tpos = tgt_pos[s, rows]
        spos = src_pos[s, rows]
        for tn, (pos, nv) in {
            "cqca": (tpos, ca_qn),
            "ckca": (spos, ca_kn),
            "cqsa": (tpos, sa_qn),
            "cksa": (tpos, sa_kn),
        }.items():
            ct, st = _rope_tables_quad(pos, nv)
            m[tn + "_c"] = ct
            m[tn + "_s"] = st
        in_maps.append(m)
    return in_maps


def _get_nc():
    if "nc" not in _cache:
        _cache["nc"] = _build_bass()
    return _cache["nc"]


def run(inputs, trace=False):
    """Run on 8 cores; returns (full_output, exec_time_ns_or_None)."""
    if trace:
        _install_ntff_hook()
    from concourse.bass_utils import run_bass_kernel_spmd

    in_maps = _prep_inputs(inputs)
    nc = _get_nc()
    res = run_bass_kernel_spmd(
        nc, in_maps, core_ids=list(range(NCORES)), trace=trace
    )
    out = np.empty((B, N, DIM), np.float32)
    for c in range(NCORES):
        s, part = c // NR, c % NR
        arr = np.asarray(res.results[c]["outT"])  # [128, 8, TOK]
        rows = slice(part * TOK, (part + 1) * TOK)
        out[s, rows] = np.transpose(arr, (2, 1, 0)).reshape(TOK, DIM)
    return out, res.exec_time_ns


def kernel(**inputs):
    out, _ = run(inputs, trace=False)
    return out


# revision 58
# speedup vs baseline: 1.0144x; 1.0006x over previous
"""Trainium2 Bass kernel for nn_CrossLayer (dense transformer layer), v5.

Sharding: sequence-parallel over 8 cores (2 samples x 4 token-chunks of 512).
Each core computes its 512 token rows through CA -> SA -> FFN.

- quad layout for q/k: head-quad tiles [128, 2, T] with partition
  p -> head 4g+(p//32), dim d = 32*j + p%32.  Rope's rotate-half becomes
  a free-dim (j) swap (no PE rotation matmul, no ACT rot copy); the cos/sin
  tables carry the qn/8 per-channel factors (host-folded).  rms-norm
  Ln/Exp batched per quad ([4,T] ACT ops).
- scores are DoubleRow matmuls on [32, 2, .] slices (explicit
  tile_position for the base-96 head); AV keeps the ones-column denominator
  trick.  Attention is fp8 e4m3 end to end; weights pre-scaled x8; /8s
  fold into the tables and the softmax-denominator exp bias (-ln 64).
- PSUM tags: "big" [128,2,512] bufs=3 (6 banks; projections, scores, FFN
  p1/p3 pairs) + "px" (2 banks, AV accumulate).  Scores triple-buffer
  against the softmax EXP on ACT so PE and ACT overlap.
- CA quad-0 k/v is computed replicated (every core does all 2048 src
  tokens) because the collective cores take ~60us to initialize after NEFF
  start; quads 1-3 AllGather in pipelined chunks that land during earlier
  groups' attention.  SA gathers all four quads (CC warm by then).
- FFN: bf16 (fp8 gives no matmul throughput on this HW and costs 1.1e-2+
  rel err); streams W1/W3 in 2-chunk tiles, hT double-buffered; the last
  quarter's residual chunks stream out as they finish.
- xT = xraw * 1/denom multiplies run on the otherwise-idle GpSimd engine.
"""

import math
import sys
import types

import numpy as np
import ml_dtypes

B, N, DIM, HID, H, D = 2, 2048, 1024, 4096, 16, 64
TOK = 512  # tokens per core
NCORES = 8
EPS = 1e-6
THETA = 10000.0
P = 128
KO = DIM // P  # 8 contraction chunks
KOP = KO // 2  # 4 DoubleRow pair-chunks
HH = H // 2  # 8 head pairs
HC = HID // P  # 32 hidden chunks
NR = 4  # ranks per replica group
SRCN = 2048  # gathered kv tokens
SKC = SRCN // P  # 16 key chunks of 128 tokens
VW = D + 1  # v columns + ones column
HG = 4  # head groups (quads: 4 heads each)
WS = 8.0  # attention weight pre-scale for fp8
LN64 = math.log(64.0)

BF = ml_dtypes.bfloat16
F8 = ml_dtypes.float8_e4m3

_cache = {}


def _quad_perm():
    """New output-channel order o' = 256*g + 128*j + p for quad layout:
    orig channel c = 64*(4g + p//32) + 32*j + (p%32)."""
    perm = np.empty(DIM, np.int64)
    for g in range(4):
        for j in range(2):
            for p in range(P):
                perm[256 * g + 128 * j + p] = (
                    64 * (4 * g + p // 32) + 32 * j + (p % 32)
                )
    return perm


_QPERM = _quad_perm()


def _lhsT_dr(W):
    """[K, M] -> [M//128, 128(K%128), K//256, 2, 128(M%128)] fp8 x8.
    Slice [mt][:, kp] is a DoubleRow lhsT [128, 2, 128]."""
    K, M = W.shape
    A = W.reshape(K // 256, 2, P, M // P, P)
    return (A.transpose(3, 2, 0, 1, 4) * WS).astype(F8).copy()


def _rhs_dr(W):
    """[K, M] -> [128, K//256, 2, M] fp8 x8 rhs-style DoubleRow moving."""
    K, M = W.shape
    A = W.reshape(K // 256, 2, P, M)
    return (A.transpose(2, 0, 1, 3) * WS).astype(F8).copy()


def _lhsT_bf(W):
    """[K, M] -> [M//128, 128(K%128), K//128, 128(M%128)] bf16 (unscaled)."""
    K, M = W.shape
    return W.reshape(K // P, P, M // P, P).transpose(2, 1, 0, 3).astype(BF).copy()


def _featmajor(x, dt):
    """[tok, dim] -> [128, dim//128, tok]."""
    n = x.shape[0]
    return x.T.reshape(DIM // P, P, n).transpose(1, 0, 2).astype(dt).copy()


def _rope_tables_quad(pos, nv):
    """pos [n] int32, nv [64] norm weights -> (cos2, sinpm) [128, 2, n] bf16.
    cos2[p,j,t] = cos(pos_t * invf[p%32]) * nv[32j + p%32] / 8
    sinpm[p,0,t] = -sin(.) * nv[32 + p%32] / 8   (d0[:,j] = v1[:,1-j]*sinpm[:,j])
    sinpm[p,1,t] = +sin(.) * nv[p%32] / 8
    """
    n = pos.shape[0]
    invf = 1.0 / (THETA ** (np.arange(0, D, 2, dtype=np.float64) / D))  # [32]
    pm32 = np.tile(np.arange(32), 4)  # p % 32 for p in 0..127
    ang = pos.astype(np.float64)[None, :] * invf[pm32][:, None]  # [128, n]
    c = np.cos(ang)
    s = np.sin(ang)
    nv = np.asarray(nv, np.float64)
    cos2 = np.empty((P, 2, n), np.float64)
    sinpm = np.empty((P, 2, n), np.float64)
    cos2[:, 0, :] = c * (nv[pm32] / WS)[:, None]
    cos2[:, 1, :] = c * (nv[32 + pm32] / WS)[:, None]
    sinpm[:, 0, :] = -s * (nv[32 + pm32] / WS)[:, None]
    sinpm[:, 1, :] = s * (nv[pm32] / WS)[:, None]
    return cos2.astype(BF).copy(), sinpm.astype(BF).copy()


def _install_ntff_hook():
    try:
        from trn_agent_boot.trn_boot import _ntff_profile_via_ctypes
    except ImportError:
        return
    if "antenv.axon_hooks" in sys.modules:
        return
    try:
        hook = _ntff_profile_via_ctypes("/opt/axon/libaxon_pjrt.so")
    except OSError:
        return
    mod = types.ModuleType("antenv.axon_hooks")
    mod.get_axon_ntff_profile_hook = lambda: hook
    mod.set_axon_ntff_profile_hook = lambda h: None
    sys.modules["antenv.axon_hooks"] = mod
    import antenv

    antenv.axon_hooks = mod


def _split_multiwait(nc):
    """This walrus only supports one sync-wait on CTRL-encoded instructions
    (Drain/NoOp); hoist excess waits onto single-wait NoOps placed before."""
    from concourse import mybir

    n_split = 0
    for f in nc.m.functions:
        for bb in f.blocks:
            new = []
            changed = False
            for ins in bb.instructions:
                si = ins.sync_info
                if (
                    si is not None
                    and si.on_wait is not None
                    and len(si.on_wait) > 1
                ):
                    waits = list(si.on_wait)
                    keep, rest = waits[:1], waits[1:]
                    for k, w in enumerate(rest):
                        new.append(
                            mybir.InstNoOp(
                                name=f"{ins.name}-wsplit{k}",
                                engine=ins.engine,
                                sync_info=mybir.SyncInfo(
                                    on_wait=[w], on_update=[]
                                ),
                                bass_nofuse=True,
                            )
                        )
                    si.on_wait = keep
                    n_split += 1
                    changed = True
                new.append(ins)
            if changed:
                bb.instructions = new
    return n_split


def _build_bass():
    from contextlib import ExitStack

    import concourse.bass as bass
    import concourse.tile as tile
    from concourse import mybir

    f32 = mybir.dt.float32
    bf16 = mybir.dt.bfloat16
    fp8 = mybir.dt.float8e4
    u8 = mybir.dt.uint8
    AF = mybir.ActivationFunctionType
    DR = mybir.MatmulPerfMode.DoubleRow
    MUL = mybir.AluOpType.mult
    ADD = mybir.AluOpType.add
    # Schraudolph-style exp straight to fp8 e4m3 bits:
    # bits = round(score * A8/8 + (56 - C8 - 3*A8)); uint8 convert saturates
    # negatives to 0.  Softmax-level error matches exact-exp + fp8 rounding.
    A8 = 8.0 / math.log(2.0)
    EXP_S1 = A8 / 8.0
    EXP_S2 = 56.0 - 0.8 - 3.0 * A8
    # DVE bit-trick exp offload disabled: a run with it produced a one-off
    # NaN (suspected dep-tracking race on the bitcast write); the ~20us gain
    # is within run noise and not worth the correctness risk.
    DVE_EXP_KC = frozenset()

    nc = bass.Bass(num_devices=NCORES)

    def inp(name, shape, dt=fp8):
        return nc.dram_tensor(name, shape, dt, kind="ExternalInput")

    tgtT = inp("tgtT", [P, KO, TOK], f32)
    srcT = inp("srcT", [P, KO, SRCN])  # full sample (CA quad-0 kv replicated)
    srcTm = inp("srcTm", [P, KO, TOK])  # this core's 512-row src shard
    # rope/norm tables: [P, 2, TOK] bf16 per (pos-set, norm-vec);
    # ckca_f covers all SRCN src positions (streamed, for replicated quad 0)
    tab_names = ["cqca", "ckca", "cqsa", "cksa"]
    tabs_in = {}
    for tn in tab_names:
        tabs_in[tn] = (
            inp(tn + "_c", [P, 2, TOK], bf16),
            inp(tn + "_s", [P, 2, TOK], bf16),
        )
    ckf_c = inp("ckf_c", [P, 2, SRCN], bf16)
    ckf_s = inp("ckf_s", [P, 2, SRCN], bf16)
    caWq = inp("caWq", [HH, P, KOP, 2, P])
    caWk = inp("caWk", [HH, P, KOP, 2, P])
    caWv = inp("caWv", [P, KOP, 2, DIM])
    caWo = inp("caWo", [KO, P, KOP, 2, P])
    saWq = inp("saWq", [HH, P, KOP, 2, P])
    saWk = inp("saWk", [HH, P, KOP, 2, P])
    saWv = inp("saWv", [P, KOP, 2, DIM])
    saWo = inp("saWo", [KO, P, KOP, 2, P])
    W1i = inp("W1", [HC, P, KO, P], bf16)
    W3i = inp("W3", [HC, P, KO, P], bf16)
    W2i = inp("W2", [KO, P, HC, P], bf16)
    blk4 = inp("blk4", [P, 4], bf16)  # per-head ssq lhsT (block ones)
    mask4 = inp("mask4", [4, P], bf16)  # rsqrt bcast lhsT (block ones)
    ones_c = inp("ones_c", [P, 1], bf16)  # y-norm ssq lhsT
    ones_r128 = inp("ones_r128", [1, P], bf16)  # y-norm bcast lhsT

    outT = nc.dram_tensor("outT", [P, KO, TOK], f32, kind="ExternalOutput")

    groups = [[0, 1, 2, 3], [4, 5, 6, 7]]
    KSZ = P * 2 * TOK  # k fp8 words per rank per head-group
    VSZ = P * 4 * 4 * VW  # v fp8 words per rank per head-group

    with tile.TileContext(nc) as tc:
        ctx = ExitStack()
        with ctx:
            sing = ctx.enter_context(tc.tile_pool(name="sing", bufs=1))
            big = ctx.enter_context(tc.tile_pool(name="big", bufs=1))
            wpool = ctx.enter_context(tc.tile_pool(name="wpool", bufs=2))
            w13p = ctx.enter_context(tc.tile_pool(name="w13p", bufs=2))
            w2p = ctx.enter_context(tc.tile_pool(name="w2p", bufs=1))
            htp = ctx.enter_context(tc.tile_pool(name="htp", bufs=2))
            work = ctx.enter_context(tc.tile_pool(name="work", bufs=2))
            probp = ctx.enter_context(tc.tile_pool(name="probp", bufs=2))
            stat = ctx.enter_context(tc.tile_pool(name="stat", bufs=2))
            dram = ctx.enter_context(
                tc.tile_pool(name="dram", bufs=1, space="DRAM")
            )
            # PSUM: "big" [128,2,512] bufs=3 (6 banks) + "px" (2 banks)
            psum = ctx.enter_context(tc.tile_pool(name="psum", bufs=3, space="PSUM"))

            def big_ps(name):
                return psum.tile([P, 2, TOK], f32, tag="big", name=name)

            def small_ps(name, part=P):
                t = psum.tile([part, 2, TOK], f32, tag="big", name=name)
                return t

            # ---- resident tiles (DMA order: CA-kv critical path first)
            blk4_sb = sing.tile([P, 4], bf16)
            nc.sync.dma_start(blk4_sb[:], blk4[:])
            mask4_sb = sing.tile([4, P], bf16)
            nc.sync.dma_start(mask4_sb[:], mask4[:])
            tabs_sb = {}

            def load_tab(tn):
                c_t, s_t = tabs_in[tn]
                cs = sing.tile([P, 2, TOK], bf16, name=tn + "_c")
                nc.sync.dma_start(cs[:], c_t[:])
                ss = sing.tile([P, 2, TOK], bf16, name=tn + "_s")
                nc.sync.dma_start(ss[:], s_t[:])
                tabs_sb[tn] = (cs, ss)

            eps_sb = sing.tile([4, 1], f32)
            nc.vector.memset(eps_sb[:], float(EPS))
            bm3_sb = sing.tile([P, 1], f32)
            nc.vector.memset(bm3_sb[:], -3.0)
            bln64_sb = sing.tile([P, 1], f32)
            nc.vector.memset(bln64_sb[:], -LN64)
            resid = sing.tile([P, KO, TOK], f32)
            nc.sync.dma_start(resid[:], tgtT[:])
            load_tab("ckca")
            ones_c_sb = sing.tile([P, 1], bf16)
            ones_r128_sb = sing.tile([1, P], bf16)

            yT = sing.tile([P, KO, TOK], fp8, name="yT")
            yF = sing.tile([P, KO, TOK], bf16, name="yF")
            q4 = sing.tile([P, HG, 2, TOK], fp8, name="q4")
            xT = sing.tile([P, HH, TOK], fp8, name="xT")
            k_mine = sing.tile([P, HG, 2, TOK], fp8, name="k_mine")
            v_mine = sing.tile([P, 4, H, VW], fp8, name="v_mine")
            nc.vector.memset(v_mine[:, :, :, D : D + 1], 1.0)
            k_full = big.tile([P, HG, 2, SRCN], fp8, tag="k_full", name="k_full")
            v_full = big.tile([P, SKC, H, VW], fp8, tag="v_full", name="v_full")
            nc.vector.memset(v_full[:, :, :, D : D + 1], 1.0)

            def proj_quad(pq, wq, ysrc):
                """8 DR matmuls: quad projection into pq [128, 2, T]."""
                for j in range(2):
                    for kp in range(KOP):
                        nc.tensor.matmul(
                            pq[:, j, :],
                            wq[:, j, kp],
                            ysrc[:, 2 * kp : 2 * kp + 2, :],
                            start=(kp == 0),
                            stop=(kp == KOP - 1),
                            perf_mode=DR,
                        )

            def norm_rope_quad(pq, cos2_sb, sinpm_sb, dst):
                """pq PSUM [128(quad), 2, T] f32 at 8x scale -> dst fp8:
                rms-normed, qn-scaled (via tables), roped (j-swap)."""
                raw = work.tile([P, 2, TOK], bf16, tag="raw", name="raw")
                nc.scalar.copy(raw[:], pq[:])
                sq = work.tile([P, 2, TOK], bf16, tag="sq", name="sq")
                nc.vector.tensor_mul(sq[:], raw[:], raw[:])
                nb = big_ps("nb")  # ssq in bank 0, bc broadcast in bank 1
                ssq = nb[0:4, 0, :]
                for j in range(2):
                    nc.tensor.matmul(
                        ssq,
                        blk4_sb[:],
                        sq[:, j, :],
                        start=(j == 0),
                        stop=(j == 1),
                    )
                # rsqrt(mean+eps) = exp(-0.5*ln(mean+eps)); 1/(64*D) unscales
                # the x8 weight prescale (squared).
                lnt = stat.tile([4, TOK], bf16, tag="lnt", name="lnt")
                nc.scalar.activation(
                    lnt[:], ssq, AF.Ln, bias=eps_sb[:], scale=1.0 / (64 * D)
                )
                rs = stat.tile([4, TOK], bf16, tag="rs", name="rs")
                nc.scalar.activation(rs[:], lnt[:], AF.Exp, scale=-0.5)
                bc = nb[:, 1, :]
                nc.tensor.matmul(bc, mask4_sb[:], rs[:], start=True, stop=True)
                v1 = work.tile([P, 2, TOK], bf16, tag="v1", name="v1")
                for j in range(2):
                    nc.vector.tensor_mul(v1[:, j, :], raw[:, j, :], bc)
                t1 = work.tile([P, 2, TOK], bf16, tag="t1", name="t1")
                nc.vector.tensor_mul(t1[:], v1[:], cos2_sb[:])
                d0 = work.tile([P, 2, TOK], bf16, tag="d0", name="d0")
                for j in range(2):
                    nc.vector.tensor_mul(
                        d0[:, j, :], v1[:, 1 - j, :], sinpm_sb[:, j, :]
                    )
                nc.vector.tensor_add(dst, t1[:], d0[:])

            def kv_group_and_ag(
                g, ysrc, Wk_t, Wv_t, tabname, kv_in, kv_out,
                kv_in_v=None, kv_out_v=None,
            ):
                """k (quad g) + v (4 heads) from my 512 rows -> AG.
                ysrc=None streams the core's src shard from DRAM.
                With kv_in_v/kv_out_v, k and v ride separate collectives
                (k first) so scores can start before v is on the wire."""
                split = kv_in_v is not None
                if ysrc is None:
                    ysrc = work.tile(
                        [P, KO, TOK], fp8, tag="srcC", bufs=2, name="srcS"
                    )
                    nc.sync.dma_start(ysrc[:], srcTm[:])
                wk = wpool.tile([P, 2, KOP, 2, P], fp8, tag="wk2", name="wk")
                nc.sync.dma_start(
                    wk[:],
                    Wk_t[2 * g : 2 * g + 2].rearrange("h p a b m -> p h a b m"),
                )
                wv = wpool.tile([P, KOP, 2, 256], fp8, tag="wv", name="wv")
                nc.sync.dma_start(
                    wv[:], Wv_t[:, :, :, g * 256 : (g + 1) * 256]
                )

                def do_k(k_dst):
                    pq = big_ps("pqk")
                    proj_quad(pq, wk, ysrc)
                    cs, ss = tabs_sb[tabname]
                    norm_rope_quad(pq, cs[:], ss[:], k_mine[:, g])
                    nc.sync.dma_start(
                        k_dst.rearrange("(p j t) -> p j t", p=P, j=2, t=TOK),
                        k_mine[:, g],
                    )

                def do_v(v_dst):
                    for t in range(4):
                        pvt = small_ps("pv")
                        pv = pvt[:, 0, 0:256]
                        for kp in range(KOP):
                            nc.tensor.matmul(
                                pv,
                                ysrc[:, 2 * kp : 2 * kp + 2, t * P : (t + 1) * P],
                                wv[:, kp],
                                start=(kp == 0),
                                stop=(kp == KOP - 1),
                                perf_mode=DR,
                            )
                        nc.vector.tensor_copy(
                            v_mine[:, t, 4 * g : 4 * g + 4, 0:D],
                            pv.rearrange("p (h d) -> p h d", d=D),
                        )
                    nc.sync.dma_start(
                        v_dst.rearrange(
                            "(p a b c) -> p a b c", p=P, a=4, b=4, c=VW
                        ),
                        v_mine[:, :, 4 * g : 4 * g + 4, :],
                    )

                def ag(in_ap, out_ap):
                    nc.gpsimd.collective_compute(
                        "AllGather",
                        mybir.AluOpType.bypass,
                        replica_groups=groups,
                        ins=[in_ap],
                        outs=[out_ap],
                    )

                if split:
                    do_k(kv_in[:])
                    ag(kv_in.opt(), kv_out.opt())
                    do_v(kv_in_v[:])
                    ag(kv_in_v.opt(), kv_out_v.opt())
                else:
                    do_v(kv_in[KSZ:])
                    do_k(kv_in[:KSZ])
                    ag(kv_in.opt(), kv_out.opt())

            def ca_rep_group(g, Wk_t, Wv_t):
                """Replicated CA kv for quad g: every core computes k/v for
                all SRCN src tokens locally (no AllGather -> no CC-init
                latency before the first CA attention group)."""
                wk = wpool.tile([P, 2, KOP, 2, P], fp8, tag="wk2", name="wkr")
                nc.sync.dma_start(
                    wk[:],
                    Wk_t[2 * g : 2 * g + 2].rearrange("h p a b m -> p h a b m"),
                )
                wv = wpool.tile([P, KOP, 2, 256], fp8, tag="wv", name="wvr")
                nc.sync.dma_start(
                    wv[:], Wv_t[:, :, :, g * 256 : (g + 1) * 256]
                )
                for c4 in range(4):
                    srcC = work.tile(
                        [P, KO, TOK], fp8, tag="srcC", bufs=2, name="srcC"
                    )
                    nc.sync.dma_start(
                        srcC[:], srcT[:, :, c4 * TOK : (c4 + 1) * TOK]
                    )
                    ckc = work.tile(
                        [P, 2, TOK], bf16, tag="ckc", bufs=1, name="ckc"
                    )
                    nc.sync.dma_start(
                        ckc[:], ckf_c[:, :, c4 * TOK : (c4 + 1) * TOK]
                    )
                    cks = work.tile(
                        [P, 2, TOK], bf16, tag="cks", bufs=1, name="cks"
                    )
                    nc.sync.dma_start(
                        cks[:], ckf_s[:, :, c4 * TOK : (c4 + 1) * TOK]
                    )
                    pq = big_ps("pqk")
                    proj_quad(pq, wk, srcC)
                    norm_rope_quad(
                        pq, ckc[:], cks[:],
                        k_full[:, g, :, c4 * TOK : (c4 + 1) * TOK],
                    )
                    # v for this chunk right away (cheap; frees srcC slot)
                    for t in range(4):
                        pvt = small_ps("pv")
                        pv = pvt[:, 0, 0:256]
                        for kp in range(KOP):
                            nc.tensor.matmul(
                                pv,
                                srcC[:, 2 * kp : 2 * kp + 2, t * P : (t + 1) * P],
                                wv[:, kp],
                                start=(kp == 0),
                                stop=(kp == KOP - 1),
                                perf_mode=DR,
                            )
                        nc.vector.tensor_copy(
                            v_full[:, 4 * c4 + t, 4 * g : 4 * g + 4, 0:D],
                            pv.rearrange("p (h d) -> p h d", d=D),
                        )

            def scatter_group(g, kv_out):
                for r in range(NR):
                    nc.sync.dma_start(
                        k_full[:, g, :, r * TOK : (r + 1) * TOK],
                        kv_out[r, :KSZ].rearrange(
                            "(p j t) -> p j t", p=P, j=2, t=TOK
                        ),
                    )
                    nc.sync.dma_start(
                        v_full[:, r * 4 : (r + 1) * 4, 4 * g : 4 * g + 4, :],
                        kv_out[r, KSZ:].rearrange(
                            "(p a b c) -> p a b c", p=P, a=4, b=4, c=VW
                        ),
                    )

            def scatter_split(g, kvo_k, kvo_v):
                for r in range(NR):
                    nc.sync.dma_start(
                        k_full[:, g, :, r * TOK : (r + 1) * TOK],
                        kvo_k[r].rearrange("(p j t) -> p j t", p=P, j=2, t=TOK),
                    )
                for r in range(NR):
                    nc.sync.dma_start(
                        v_full[:, r * 4 : (r + 1) * 4, 4 * g : 4 * g + 4, :],
                        kvo_v[r].rearrange(
                            "(p a b c) -> p a b c", p=P, a=4, b=4, c=VW
                        ),
                    )

            def proj_q(Wt, tabname, quads=range(HG)):
                """y -> q (given quads), normed+roped into q4."""
                for g in quads:
                    wq = wpool.tile([P, 2, KOP, 2, P], fp8, tag="wk2", name="wq")
                    nc.sync.dma_start(
                        wq[:],
                        Wt[2 * g : 2 * g + 2].rearrange("h p a b m -> p h a b m"),
                    )
                    pq = big_ps("pq")
                    proj_quad(pq, wq, yT)
                    cs, ss = tabs_sb[tabname]
                    norm_rope_quad(pq, cs[:], ss[:], q4[:, g])

            def attention_group(hg, kdb):
                """scores+softmax+AV for quad hg (pairs j=0,1); fills
                xT[:, 2hg:2hg+2] with x_norm/8 (fp8)."""
                xraw = stat.tile(
                    [P, 2, TOK], bf16, tag="xraw", bufs=1, name="xraw"
                )
                dens4 = work.tile(
                    [P, 4, TOK], bf16, tag="dens", bufs=1, name="dens4"
                )
                for j in range(2):
                    px = psum.tile(
                        [VW, 2, TOK], f32, tag="px", bufs=1, name="px"
                    )
                    for kc in range(SKC):
                        ps = big_ps("ps")
                        for i in range(2):
                            b = 2 * j + i
                            nc.tensor.matmul(
                                ps[:, i, :],
                                k_full[
                                    32 * b : 32 * b + 32,
                                    hg,
                                    :,
                                    kc * P : (kc + 1) * P,
                                ],
                                q4[32 * b : 32 * b + 32, hg],
                                start=True,
                                stop=True,
                                perf_mode=DR,
                                tile_position=(32 * b, 0),
                            )
                        if kc % 2 == 0:
                            prob = probp.tile(
                                [P, 2, 2, TOK], fp8, tag="prob", name="prob"
                            )
                        if kc in DVE_EXP_KC:
                            nc.vector.tensor_scalar(
                                prob[:, kc % 2].bitcast(u8), ps[:],
                                EXP_S1, EXP_S2, MUL, ADD,
                            )
                        else:
                            nc.scalar.activation(
                                prob[:, kc % 2],
                                ps[:],
                                AF.Exp,
                                scale=1.0 / math.sqrt(D),
                                bias=bm3_sb[:],
                            )
                        if kc % 2 == 1:
                            j2 = kc - 1
                            for i in range(2):
                                nc.tensor.matmul(
                                    px[:, i, :],
                                    v_full[:, j2 : j2 + 2, hg * 4 + 2 * j + i, :],
                                    prob[:, :, i, :],
                                    start=(kc == 1),
                                    stop=(kc == SKC - 1),
                                    perf_mode=DR,
                                )
                    for i in range(2):
                        # denom row rides on partition 64 (ones column of v)
                        nc.vector.tensor_copy(
                            dens4[D : D + 1, 2 * j + i], px[D : D + 1, i, :]
                        )
                        nc.vector.tensor_copy(
                            xraw[i * D : (i + 1) * D, j], px[0:D, i, :]
                        )
                # reciprocals: 1/(64*den); the 64 unscales v and Wo x8 each,
                # making xT = x_norm/8 which Wo's x8 restores
                nc.sync.dma_start(
                    kdb[: 4 * TOK].rearrange("(o f t) -> o f t", o=1, f=4),
                    dens4[D : D + 1],
                )
                d4 = stat.tile([4, TOK], bf16, tag="d4", bufs=1, name="d4")
                nc.sync.dma_start(
                    d4[:], kdb[: 4 * TOK].rearrange("(f t) -> f t", f=4)
                )
                nc.scalar.activation(d4[:], d4[:], AF.Ln)
                rec4 = stat.tile([4, TOK], bf16, tag="rec4", bufs=1, name="rec4")
                nc.scalar.activation(
                    rec4[:], d4[:], AF.Exp, scale=-1.0, bias=bln64_sb[:4]
                )
                nc.sync.dma_start(
                    kdb[4 * TOK :].rearrange("(f t) -> f t", f=4), rec4[:]
                )
                rec_bc = work.tile(
                    [P, 2, TOK], bf16, tag="recbc", bufs=1, name="rec_bc"
                )
                for i in range(2):
                    src = bass.AP(
                        tensor=kdb.tensor,
                        offset=kdb.offset + 4 * TOK + i * TOK,
                        ap=[[0, D], [2 * TOK, 2], [1, TOK]],
                    )
                    nc.sync.dma_start(rec_bc[i * D : (i + 1) * D], src)
                for j in range(2):
                    nc.gpsimd.tensor_mul(
                        xT[:, 2 * hg + j], xraw[:, j], rec_bc[:, j]
                    )

            def wo_group(hg, Wo_t):
                """Wo partial for head-quad hg, accumulated into resid."""
                wo = wpool.tile([P, KO, 2, P], fp8, tag="wo", name="wo")
                nc.sync.dma_start(
                    wo[:], Wo_t[:, :, hg].rearrange("o p b m -> p o b m")
                )
                for oc in range(KO):
                    pot = small_ps("po")
                    po = pot[:, 0, :]
                    nc.tensor.matmul(
                        po,
                        wo[:, oc],
                        xT[:, 2 * hg : 2 * hg + 2, :],
                        start=True,
                        stop=True,
                        perf_mode=DR,
                    )
                    nc.vector.tensor_add(resid[:, oc], resid[:, oc], po)

            def rmsnorm_feat(dst):
                """resid f32 -> dst (fp8 or bf16): resid * rsqrt(mean sq)."""
                nb = big_ps("ynb")  # ssq in bank 0, bc broadcast in bank 1
                ssq = nb[0:1, 0, :]
                for c in range(KO):
                    sq = work.tile([P, TOK], bf16, tag="ysq", name="ynsq")
                    nc.vector.tensor_mul(sq[:], resid[:, c], resid[:, c])
                    nc.tensor.matmul(
                        ssq,
                        ones_c_sb[:],
                        sq[:],
                        start=(c == 0),
                        stop=(c == KO - 1),
                    )
                lnt = stat.tile([1, TOK], bf16, tag="lnt", name="ylnt")
                nc.scalar.activation(
                    lnt[:], ssq, AF.Ln, bias=eps_sb[:1], scale=1.0 / DIM
                )
                rs = stat.tile([1, TOK], bf16, tag="rs", name="yrs")
                nc.scalar.activation(rs[:], lnt[:], AF.Exp, scale=-0.5)
                bc = nb[:, 1, :]
                nc.tensor.matmul(bc, ones_r128_sb[:], rs[:], start=True, stop=True)
                for c in range(KO):
                    nc.vector.tensor_mul(dst[:, c], resid[:, c], bc)

            # ================= cross-attention =================
            # quad 0's kv is computed locally on every core (hides the CC
            # init latency); quads 1-3 go through pipelined AllGathers.
            kvi_ca = [
                dram.tile([KSZ + VSZ], fp8, tag=f"kvica{g}", name=f"kvica{g}")
                for g in range(1, HG)
            ]
            kvo_ca = [
                dram.tile([NR, KSZ + VSZ], fp8, tag=f"kvoca{g}", name=f"kvoca{g}")
                for g in range(1, HG)
            ]
            kdbs = [
                dram.tile([8 * TOK], bf16, tag=f"kdb{g}", name=f"kdb{g}")
                for g in range(HG)
            ]
            nc.sync.dma_start(ones_c_sb[:], ones_c[:])
            nc.sync.dma_start(ones_r128_sb[:], ones_r128[:])
            load_tab("cqca")
            rmsnorm_feat(yT)
            proj_q(caWq, "cqca")
            ca_rep_group(0, caWk, caWv)
            kv_group_and_ag(1, None, caWk, caWv, "ckca", kvi_ca[0], kvo_ca[0])
            load_tab("cqsa")
            load_tab("cksa")
            attention_group(0, kdbs[0])
            kv_group_and_ag(2, None, caWk, caWv, "ckca", kvi_ca[1], kvo_ca[1])
            scatter_group(1, kvo_ca[0])
            attention_group(1, kdbs[1])
            wo_group(0, caWo)
            kv_group_and_ag(3, None, caWk, caWv, "ckca", kvi_ca[2], kvo_ca[2])
            scatter_group(2, kvo_ca[1])
            attention_group(2, kdbs[2])
            wo_group(1, caWo)
            scatter_group(3, kvo_ca[2])
            attention_group(3, kdbs[3])
            wo_group(2, caWo)
            wo_group(3, caWo)

            # ================= self-attention =================
            rmsnorm_feat(yT)
            kvik_sa = [
                dram.tile([KSZ], fp8, tag=f"kviksa{g}", name=f"kviksa{g}")
                for g in range(HG)
            ]
            kvok_sa = [
                dram.tile([NR, KSZ], fp8, tag=f"kvoksa{g}", name=f"kvoksa{g}")
                for g in range(HG)
            ]
            kviv_sa = [
                dram.tile([VSZ], fp8, tag=f"kvivsa{g}", name=f"kvivsa{g}")
                for g in range(HG)
            ]
            kvov_sa = [
                dram.tile([NR, VSZ], fp8, tag=f"kvovsa{g}", name=f"kvovsa{g}")
                for g in range(HG)
            ]
            kdbs2 = [
                dram.tile([8 * TOK], bf16, tag=f"kdc{g}", name=f"kdc{g}")
                for g in range(HG)
            ]
            for g in range(HG):
                kv_group_and_ag(
                    g, yT, saWk, saWv, "cksa", kvik_sa[g], kvok_sa[g],
                    kviv_sa[g], kvov_sa[g],
                )
                if g == 0:
                    proj_q(saWq, "cqsa")
            for hg in range(HG):
                scatter_split(hg, kvok_sa[hg], kvov_sa[hg])
                attention_group(hg, kdbs2[hg])
                if hg >= 1:
                    wo_group(hg - 1, saWo)
            wo_group(HG - 1, saWo)

            # ================= FFN (bf16) =================
            rmsnorm_feat(yF)
            for qtr in range(4):
                hT = htp.tile([P, 8, TOK], bf16, tag="hT", name="hT")
                for e in range(4):
                    w1 = w13p.tile([P, 2, KO, P], bf16, tag="w1", name="w1")
                    nc.sync.dma_start(
                        w1[:],
                        W1i[qtr * 8 + e * 2 : qtr * 8 + e * 2 + 2].rearrange(
                            "h p a m -> p h a m"
                        ),
                    )
                    w3 = w13p.tile([P, 2, KO, P], bf16, tag="w3", name="w3")
                    nc.sync.dma_start(
                        w3[:],
                        W3i[qtr * 8 + e * 2 : qtr * 8 + e * 2 + 2].rearrange(
                            "h p a m -> p h a m"
                        ),
                    )
                    for gg in range(2):
                        p13 = big_ps("p13")
                        p1 = p13[:, 0, :]
                        p3 = p13[:, 1, :]
                        for c in range(KO):
                            nc.tensor.matmul(
                                p1, w1[:, gg, c], yF[:, c],
                                start=(c == 0), stop=(c == KO - 1),
                            )
                        for c in range(KO):
                            nc.tensor.matmul(
                                p3, w3[:, gg, c], yF[:, c],
                                start=(c == 0), stop=(c == KO - 1),
                            )
                        s1 = stat.tile([P, TOK], bf16, tag="s1", name="s1")
                        nc.scalar.activation(s1[:], p1, AF.Silu)
                        nc.vector.tensor_mul(hT[:, e * 2 + gg], s1[:], p3)
                w2 = w2p.tile([P, KO, 8, P], bf16, tag="w2", name="w2")
                nc.sync.dma_start(
                    w2[:],
                    W2i[:, :, qtr * 8 : (qtr + 1) * 8].rearrange(
                        "o p a m -> p o a m"
                    ),
                )
                for oc in range(KO):
                    pot = small_ps("po2")
                    po = pot[:, 0, :]
                    for gg in range(8):
                        nc.tensor.matmul(
                            po, w2[:, oc, gg], hT[:, gg],
                            start=(gg == 0), stop=(gg == 7),
                        )
                    nc.vector.tensor_add(resid[:, oc], resid[:, oc], po)
                    if qtr == 3:
                        nc.sync.dma_start(outT[:, oc], resid[:, oc])

    _split_multiwait(nc)
    return nc


def _prep_inputs(inputs):
    """Full problem inputs -> list of 8 per-core in_maps."""
    tgt = np.asarray(inputs["tgt"], np.float32)
    src = np.asarray(inputs["src"], np.float32)
    tgt_pos = np.asarray(inputs["tgt_pos"], np.int32)
    src_pos = np.asarray(inputs["src_pos"], np.int32)

    pre_ca_w = np.asarray(inputs["pre_ca_w"], np.float32)
    pre_sa_w = np.asarray(inputs["pre_sa_w"], np.float32)
    pre_ffn_w = np.asarray(inputs["pre_ffn_w"], np.float32)

    def fold(Wname, w):
        return np.asarray(inputs[Wname], np.float32) * w[:, None]

    ca_Wq = fold("ca_Wq", pre_ca_w)
    ca_Wkv = np.asarray(inputs["ca_Wkv"], np.float32)
    ca_Wk, ca_Wv = ca_Wkv[:, :DIM], ca_Wkv[:, DIM:]
    ca_Wo = np.asarray(inputs["ca_Wo"], np.float32)
    sa_Wq = fold("sa_Wq", pre_sa_w)
    sa_Wkv = fold("sa_Wkv", pre_sa_w)
    sa_Wk, sa_Wv = sa_Wkv[:, :DIM], sa_Wkv[:, DIM:]
    sa_Wo = np.asarray(inputs["sa_Wo"], np.float32)
    W1 = fold("W1", pre_ffn_w)
    W3 = fold("W3", pre_ffn_w)
    W2 = np.asarray(inputs["W2"], np.float32)

    shared = {
        "caWq": _lhsT_dr(ca_Wq[:, _QPERM]),
        "caWk": _lhsT_dr(ca_Wk[:, _QPERM]),
        "caWv": _rhs_dr(ca_Wv),
        "caWo": _lhsT_dr(ca_Wo),
        "saWq": _lhsT_dr(sa_Wq[:, _QPERM]),
        "saWk": _lhsT_dr(sa_Wk[:, _QPERM]),
        "saWv": _rhs_dr(sa_Wv),
        "saWo": _lhsT_dr(sa_Wo),
        "W1": _lhsT_bf(W1),
        "W3": _lhsT_bf(W3),
        "W2": _lhsT_bf(W2),
    }

    blk4 = np.zeros((P, 4), np.float32)
    for m in range(4):
        blk4[32 * m : 32 * m + 32, m] = 1
    shared["blk4"] = blk4.astype(BF).copy()
    shared["mask4"] = blk4.T.astype(BF).copy()
    shared["ones_c"] = np.ones((P, 1), BF)
    shared["ones_r128"] = np.ones((1, P), BF)

    ca_qn = np.asarray(inputs["ca_qn"], np.float32)
    ca_kn = np.asarray(inputs["ca_kn"], np.float32)
    sa_qn = np.asarray(inputs["sa_qn"], np.float32)
    sa_kn = np.asarray(inputs["sa_kn"], np.float32)

    srcT_full = [_featmajor(src[s], F8) for s in range(B)]
    ckf = [_rope_tables_quad(src_pos[s], ca_kn) for s in range(B)]

    in_maps = []
    for c in range(NCORES):
        s, part = c // NR, c % NR
        rows = slice(part * TOK, (part + 1) * TOK)
        m = dict(shared)
        m["tgtT"] = _featmajor(tgt[s, rows], np.float32)
        m["srcT"] = srcT_full[s]
        m["srcTm"] = _featmajor(src[s, rows], F8)
        m["ckf_c"], m["ckf_s"] = ckf[s]
        tpos = tgt_pos[s, rows]
        spos = src_pos[s, rows]
        for tn, (pos, nv) in {
            "cqca": (tpos, ca_qn),
            "ckca": (spos, ca_kn),
            "cqsa": (tpos, sa_qn),
            "cksa": (tpos, sa_kn),
        }.items():
            ct, st = _rope_tables_quad(pos, nv)
            m[tn + "_c"] = ct
            m[tn + "_s"] = st
        in_maps.append(m)
    return in_maps


def _get_nc():
    if "nc" not in _cache:
        _cache["nc"] = _build_bass()
    return _cache["nc"]


def run(inputs, trace=False):
    """Run on 8 cores; returns (full_output, exec_time_ns_or_None)."""
    if trace:
        _install_ntff_hook()
    from concourse.bass_utils import run_bass_kernel_spmd

    in_maps = _prep_inputs(inputs)
    nc = _get_nc()
    res = run_bass_kernel_spmd(
        nc, in_maps, core_ids=list(range(NCORES)), trace=trace
    )
    out = np.empty((B, N, DIM), np.float32)
    for c in range(NCORES):
        s, part = c // NR, c % NR
        arr = np.asarray(res.results[c]["outT"])  # [128, 8, TOK]
        rows = slice(part * TOK, (part + 1) * TOK)
        out[s, rows] = np.transpose(arr, (2, 1, 0)).reshape(TOK, DIM)
    return out, res.exec_time_ns


def kernel(**inputs):
    out, _ = run(inputs, trace=False)
    return out


# revision 59
# speedup vs baseline: 1.0491x; 1.0342x over previous
"""Trainium2 Bass kernel for nn_CrossLayer (dense transformer layer), v5.

Sharding: sequence-parallel over 8 cores (2 samples x 4 token-chunks of 512).
Each core computes its 512 token rows through CA -> SA -> FFN.

- quad layout for q/k: head-quad tiles [128, 2, T] with partition
  p -> head 4g+(p//32), dim d = 32*j + p%32.  Rope's rotate-half becomes
  a free-dim (j) swap (no PE rotation matmul, no ACT rot copy); the cos/sin
  tables carry the qn/8 per-channel factors (host-folded).  rms-norm
  Ln/Exp batched per quad ([4,T] ACT ops).
- scores are DoubleRow matmuls on [32, 2, .] slices (explicit
  tile_position for the base-96 head); AV keeps the ones-column denominator
  trick.  Attention is fp8 e4m3 end to end; weights pre-scaled x8; /8s
  fold into the tables and the softmax-denominator exp bias (-ln 64).
- PSUM tags: "big" [128,2,512] bufs=3 (6 banks; projections, scores, FFN
  p1/p3 pairs) + "px" (2 banks, AV accumulate).  Scores triple-buffer
  against the softmax EXP on ACT so PE and ACT overlap.
- CA quad-0 k/v is computed replicated (every core does all 2048 src
  tokens) because the collective cores take ~60us to initialize after NEFF
  start; quads 1-3 AllGather in pipelined chunks that land during earlier
  groups' attention.  SA gathers all four quads (CC warm by then).
- FFN: bf16 (fp8 gives no matmul throughput on this HW and costs 1.1e-2+
  rel err); streams W1/W3 in 2-chunk tiles, hT double-buffered; the last
  quarter's residual chunks stream out as they finish.
- xT = xraw * 1/denom multiplies run on the otherwise-idle GpSimd engine.
"""

import math
import sys
import types

import numpy as np
import ml_dtypes

B, N, DIM, HID, H, D = 2, 2048, 1024, 4096, 16, 64
TOK = 512  # tokens per core
NCORES = 8
EPS = 1e-6
THETA = 10000.0
P = 128
KO = DIM // P  # 8 contraction chunks
KOP = KO // 2  # 4 DoubleRow pair-chunks
HH = H // 2  # 8 head pairs
HC = HID // P  # 32 hidden chunks
NR = 4  # ranks per replica group
SRCN = 2048  # gathered kv tokens
SKC = SRCN // P  # 16 key chunks of 128 tokens
VW = D + 1  # v columns + ones column
HG = 4  # head groups (quads: 4 heads each)
WS = 8.0  # attention weight pre-scale for fp8
LN64 = math.log(64.0)

BF = ml_dtypes.bfloat16
F8 = ml_dtypes.float8_e4m3

_cache = {}


def _quad_perm():
    """New output-channel order o' = 256*g + 128*j + p for quad layout:
    orig channel c = 64*(4g + p//32) + 32*j + (p%32)."""
    perm = np.empty(DIM, np.int64)
    for g in range(4):
        for j in range(2):
            for p in range(P):
                perm[256 * g + 128 * j + p] = (
                    64 * (4 * g + p // 32) + 32 * j + (p % 32)
                )
    return perm


_QPERM = _quad_perm()


def _lhsT_dr(W):
    """[K, M] -> [M//128, 128(K%128), K//256, 2, 128(M%128)] fp8 x8.
    Slice [mt][:, kp] is a DoubleRow lhsT [128, 2, 128]."""
    K, M = W.shape
    A = W.reshape(K // 256, 2, P, M // P, P)
    return (A.transpose(3, 2, 0, 1, 4) * WS).astype(F8).copy()


def _rhs_dr(W):
    """[K, M] -> [128, K//256, 2, M] fp8 x8 rhs-style DoubleRow moving."""
    K, M = W.shape
    A = W.reshape(K // 256, 2, P, M)
    return (A.transpose(2, 0, 1, 3) * WS).astype(F8).copy()


def _lhsT_bf(W):
    """[K, M] -> [M//128, 128(K%128), K//128, 128(M%128)] bf16 (unscaled)."""
    K, M = W.shape
    return W.reshape(K // P, P, M // P, P).transpose(2, 1, 0, 3).astype(BF).copy()


def _featmajor(x, dt):
    """[tok, dim] -> [128, dim//128, tok]."""
    n = x.shape[0]
    return x.T.reshape(DIM // P, P, n).transpose(1, 0, 2).astype(dt).copy()


def _rope_tables_quad(pos, nv):
    """pos [n] int32, nv [64] norm weights -> (cos2, sinpm) [128, 2, n] bf16.
    cos2[p,j,t] = cos(pos_t * invf[p%32]) * nv[32j + p%32] / 8
    sinpm[p,0,t] = -sin(.) * nv[32 + p%32] / 8   (d0[:,j] = v1[:,1-j]*sinpm[:,j])
    sinpm[p,1,t] = +sin(.) * nv[p%32] / 8
    """
    n = pos.shape[0]
    invf = 1.0 / (THETA ** (np.arange(0, D, 2, dtype=np.float64) / D))  # [32]
    pm32 = np.tile(np.arange(32), 4)  # p % 32 for p in 0..127
    ang = pos.astype(np.float64)[None, :] * invf[pm32][:, None]  # [128, n]
    c = np.cos(ang)
    s = np.sin(ang)
    nv = np.asarray(nv, np.float64)
    cos2 = np.empty((P, 2, n), np.float64)
    sinpm = np.empty((P, 2, n), np.float64)
    cos2[:, 0, :] = c * (nv[pm32] / WS)[:, None]
    cos2[:, 1, :] = c * (nv[32 + pm32] / WS)[:, None]
    sinpm[:, 0, :] = -s * (nv[32 + pm32] / WS)[:, None]
    sinpm[:, 1, :] = s * (nv[pm32] / WS)[:, None]
    return cos2.astype(BF).copy(), sinpm.astype(BF).copy()


def _install_ntff_hook():
    try:
        from trn_agent_boot.trn_boot import _ntff_profile_via_ctypes
    except ImportError:
        return
    if "antenv.axon_hooks" in sys.modules:
        return
    try:
        hook = _ntff_profile_via_ctypes("/opt/axon/libaxon_pjrt.so")
    except OSError:
        return
    mod = types.ModuleType("antenv.axon_hooks")
    mod.get_axon_ntff_profile_hook = lambda: hook
    mod.set_axon_ntff_profile_hook = lambda h: None
    sys.modules["antenv.axon_hooks"] = mod
    import antenv

    antenv.axon_hooks = mod


def _split_multiwait(nc):
    """This walrus only supports one sync-wait on CTRL-encoded instructions
    (Drain/NoOp); hoist excess waits onto single-wait NoOps placed before."""
    from concourse import mybir

    n_split = 0
    for f in nc.m.functions:
        for bb in f.blocks:
            new = []
            changed = False
            for ins in bb.instructions:
                si = ins.sync_info
                if (
                    si is not None
                    and si.on_wait is not None
                    and len(si.on_wait) > 1
                ):
                    waits = list(si.on_wait)
                    keep, rest = waits[:1], waits[1:]
                    for k, w in enumerate(rest):
                        new.append(
                            mybir.InstNoOp(
                                name=f"{ins.name}-wsplit{k}",
                                engine=ins.engine,
                                sync_info=mybir.SyncInfo(
                                    on_wait=[w], on_update=[]
                                ),
                                bass_nofuse=True,
                            )
                        )
                    si.on_wait = keep
                    n_split += 1
                    changed = True
                new.append(ins)
            if changed:
                bb.instructions = new
    return n_split


def _build_bass():
    from contextlib import ExitStack

    import concourse.bass as bass
    import concourse.tile as tile
    from concourse import mybir

    f32 = mybir.dt.float32
    bf16 = mybir.dt.bfloat16
    fp8 = mybir.dt.float8e4
    u8 = mybir.dt.uint8
    AF = mybir.ActivationFunctionType
    DR = mybir.MatmulPerfMode.DoubleRow
    MUL = mybir.AluOpType.mult
    ADD = mybir.AluOpType.add
    # Schraudolph-style exp straight to fp8 e4m3 bits:
    # bits = round(score * A8/8 + (56 - C8 - 3*A8)); uint8 convert saturates
    # negatives to 0.  Softmax-level error matches exact-exp + fp8 rounding.
    A8 = 8.0 / math.log(2.0)
    EXP_S1 = A8 / 8.0
    EXP_S2 = 56.0 - 0.8 - 3.0 * A8
    # DVE bit-trick exp offload disabled: a run with it produced a one-off
    # NaN (suspected dep-tracking race on the bitcast write); the ~20us gain
    # is within run noise and not worth the correctness risk.
    DVE_EXP_KC = frozenset()

    nc = bass.Bass(num_devices=NCORES)

    def inp(name, shape, dt=fp8):
        return nc.dram_tensor(name, shape, dt, kind="ExternalInput")

    tgtT = inp("tgtT", [P, KO, TOK], f32)
    srcT = inp("srcT", [P, KO, SRCN])  # full sample (CA quad-0 kv replicated)
    srcTm = inp("srcTm", [P, KO, TOK])  # this core's 512-row src shard
    # rope/norm tables: [P, 2, TOK] bf16 per (pos-set, norm-vec);
    # ckca_f covers all SRCN src positions (streamed, for replicated quad 0)
    tab_names = ["cqca", "ckca", "cqsa", "cksa"]
    tabs_in = {}
    for tn in tab_names:
        tabs_in[tn] = (
            inp(tn + "_c", [P, 2, TOK], bf16),
            inp(tn + "_s", [P, 2, TOK], bf16),
        )
    ckf_c = inp("ckf_c", [P, 2, SRCN], bf16)
    ckf_s = inp("ckf_s", [P, 2, SRCN], bf16)
    caWq = inp("caWq", [HH, P, KOP, 2, P])
    caWk = inp("caWk", [HH, P, KOP, 2, P])
    caWv = inp("caWv", [P, KOP, 2, DIM])
    caWo = inp("caWo", [KO, P, KOP, 2, P])
    saWq = inp("saWq", [HH, P, KOP, 2, P])
    saWk = inp("saWk", [HH, P, KOP, 2, P])
    saWv = inp("saWv", [P, KOP, 2, DIM])
    saWo = inp("saWo", [KO, P, KOP, 2, P])
    W1i = inp("W1", [HC, P, KO, P], bf16)
    W3i = inp("W3", [HC, P, KO, P], bf16)
    W2i = inp("W2", [KO, P, HC, P], bf16)
    blk4 = inp("blk4", [P, 4], bf16)  # per-head ssq lhsT (block ones)
    mask4 = inp("mask4", [4, P], bf16)  # rsqrt bcast lhsT (block ones)
    ones_c = inp("ones_c", [P, 1], bf16)  # y-norm ssq lhsT
    ones_r128 = inp("ones_r128", [1, P], bf16)  # y-norm bcast lhsT

    outT = nc.dram_tensor("outT", [P, KO, TOK], f32, kind="ExternalOutput")

    groups = [[0, 1, 2, 3], [4, 5, 6, 7]]
    KSZ = P * 2 * TOK  # k fp8 words per rank per head-group
    VSZ = P * 4 * 4 * VW  # v fp8 words per rank per head-group

    with tile.TileContext(nc) as tc:
        ctx = ExitStack()
        with ctx:
            sing = ctx.enter_context(tc.tile_pool(name="sing", bufs=1))
            big = ctx.enter_context(tc.tile_pool(name="big", bufs=1))
            wpool = ctx.enter_context(tc.tile_pool(name="wpool", bufs=2))
            w13p = ctx.enter_context(tc.tile_pool(name="w13p", bufs=2))
            w2p = ctx.enter_context(tc.tile_pool(name="w2p", bufs=1))
            htp = ctx.enter_context(tc.tile_pool(name="htp", bufs=2))
            work = ctx.enter_context(tc.tile_pool(name="work", bufs=2))
            probp = ctx.enter_context(tc.tile_pool(name="probp", bufs=2))
            stat = ctx.enter_context(tc.tile_pool(name="stat", bufs=2))
            dram = ctx.enter_context(
                tc.tile_pool(name="dram", bufs=1, space="DRAM")
            )
            # PSUM: "big" [128,2,512] bufs=3 (6 banks) + "px" (2 banks)
            psum = ctx.enter_context(tc.tile_pool(name="psum", bufs=3, space="PSUM"))

            def big_ps(name):
                return psum.tile([P, 2, TOK], f32, tag="big", name=name)

            def small_ps(name, part=P):
                t = psum.tile([part, 2, TOK], f32, tag="big", name=name)
                return t

            # ---- resident tiles (DMA order: CA-kv critical path first)
            blk4_sb = sing.tile([P, 4], bf16)
            nc.sync.dma_start(blk4_sb[:], blk4[:])
            mask4_sb = sing.tile([4, P], bf16)
            nc.sync.dma_start(mask4_sb[:], mask4[:])
            tabs_sb = {}

            def load_tab(tn):
                c_t, s_t = tabs_in[tn]
                cs = sing.tile([P, 2, TOK], bf16, name=tn + "_c")
                nc.sync.dma_start(cs[:], c_t[:])
                ss = sing.tile([P, 2, TOK], bf16, name=tn + "_s")
                nc.sync.dma_start(ss[:], s_t[:])
                tabs_sb[tn] = (cs, ss)

            eps_sb = sing.tile([4, 1], f32)
            nc.vector.memset(eps_sb[:], float(EPS))
            bm3_sb = sing.tile([P, 1], f32)
            nc.vector.memset(bm3_sb[:], -3.0)
            bln64_sb = sing.tile([P, 1], f32)
            nc.vector.memset(bln64_sb[:], -LN64)
            resid = sing.tile([P, KO, TOK], f32)
            nc.sync.dma_start(resid[:], tgtT[:])
            load_tab("ckca")
            ones_c_sb = sing.tile([P, 1], bf16)
            ones_r128_sb = sing.tile([1, P], bf16)

            yT = sing.tile([P, KO, TOK], fp8, name="yT")
            yF = sing.tile([P, KO, TOK], bf16, name="yF")
            q4 = sing.tile([P, HG, 2, TOK], fp8, name="q4")
            xT = sing.tile([P, HH, TOK], fp8, name="xT")
            k_mine = sing.tile([P, HG, 2, TOK], fp8, name="k_mine")
            v_mine = sing.tile([P, 4, H, VW], fp8, name="v_mine")
            nc.vector.memset(v_mine[:, :, :, D : D + 1], 1.0)
            k_full = big.tile([P, HG, 2, SRCN], fp8, tag="k_full", name="k_full")
            v_full = big.tile([P, SKC, H, VW], fp8, tag="v_full", name="v_full")
            nc.vector.memset(v_full[:, :, :, D : D + 1], 1.0)

            def proj_quad(pq, wq, ysrc):
                """8 DR matmuls: quad projection into pq [128, 2, T]."""
                for j in range(2):
                    for kp in range(KOP):
                        nc.tensor.matmul(
                            pq[:, j, :],
                            wq[:, j, kp],
                            ysrc[:, 2 * kp : 2 * kp + 2, :],
                            start=(kp == 0),
                            stop=(kp == KOP - 1),
                            perf_mode=DR,
                        )

            def norm_rope_quad(pq, cos2_sb, sinpm_sb, dst):
                """pq PSUM [128(quad), 2, T] f32 at 8x scale -> dst fp8:
                rms-normed, qn-scaled (via tables), roped (j-swap)."""
                raw = work.tile([P, 2, TOK], bf16, tag="raw", name="raw")
                nc.scalar.copy(raw[:], pq[:])
                sq = work.tile([P, 2, TOK], bf16, tag="sq", name="sq")
                nc.vector.tensor_mul(sq[:], raw[:], raw[:])
                nb = big_ps("nb")  # ssq in bank 0, bc broadcast in bank 1
                ssq = nb[0:4, 0, :]
                for j in range(2):
                    nc.tensor.matmul(
                        ssq,
                        blk4_sb[:],
                        sq[:, j, :],
                        start=(j == 0),
                        stop=(j == 1),
                    )
                # rsqrt(mean+eps) = exp(-0.5*ln(mean+eps)); 1/(64*D) unscales
                # the x8 weight prescale (squared).
                lnt = stat.tile([4, TOK], bf16, tag="lnt", name="lnt")
                nc.scalar.activation(
                    lnt[:], ssq, AF.Ln, bias=eps_sb[:], scale=1.0 / (64 * D)
                )
                rs = stat.tile([4, TOK], bf16, tag="rs", name="rs")
                nc.scalar.activation(rs[:], lnt[:], AF.Exp, scale=-0.5)
                bc = nb[:, 1, :]
                nc.tensor.matmul(bc, mask4_sb[:], rs[:], start=True, stop=True)
                v1 = work.tile([P, 2, TOK], bf16, tag="v1", name="v1")
                for j in range(2):
                    nc.vector.tensor_mul(v1[:, j, :], raw[:, j, :], bc)
                t1 = work.tile([P, 2, TOK], bf16, tag="t1", name="t1")
                nc.vector.tensor_mul(t1[:], v1[:], cos2_sb[:])
                d0 = work.tile([P, 2, TOK], bf16, tag="d0", name="d0")
                for j in range(2):
                    nc.vector.tensor_mul(
                        d0[:, j, :], v1[:, 1 - j, :], sinpm_sb[:, j, :]
                    )
                nc.vector.tensor_add(dst, t1[:], d0[:])

            def kv_group_and_ag(
                g, ysrc, Wk_t, Wv_t, tabname, kv_in, kv_out, split=False
            ):
                """k (quad g) + v (4 heads) from my 512 rows -> AG.
                ysrc=None streams the core's src shard from DRAM.
                split=True gathers k first in its own collective so scores
                can start before v is on the wire."""
                if ysrc is None:
                    ysrc = work.tile(
                        [P, KO, TOK], fp8, tag="srcC", bufs=2, name="srcS"
                    )
                    nc.sync.dma_start(ysrc[:], srcTm[:])
                wk = wpool.tile([P, 2, KOP, 2, P], fp8, tag="wk2", name="wk")
                nc.sync.dma_start(
                    wk[:],
                    Wk_t[2 * g : 2 * g + 2].rearrange("h p a b m -> p h a b m"),
                )
                wv = wpool.tile([P, KOP, 2, 256], fp8, tag="wv", name="wv")
                nc.sync.dma_start(
                    wv[:], Wv_t[:, :, :, g * 256 : (g + 1) * 256]
                )

                def do_k():
                    pq = big_ps("pqk")
                    proj_quad(pq, wk, ysrc)
                    cs, ss = tabs_sb[tabname]
                    norm_rope_quad(pq, cs[:], ss[:], k_mine[:, g])
                    nc.sync.dma_start(
                        kv_in[:KSZ].rearrange(
                            "(p j t) -> p j t", p=P, j=2, t=TOK
                        ),
                        k_mine[:, g],
                    )

                def do_v():
                    for t in range(4):
                        pvt = small_ps("pv")
                        pv = pvt[:, 0, 0:256]
                        for kp in range(KOP):
                            nc.tensor.matmul(
                                pv,
                                ysrc[:, 2 * kp : 2 * kp + 2, t * P : (t + 1) * P],
                                wv[:, kp],
                                start=(kp == 0),
                                stop=(kp == KOP - 1),
                                perf_mode=DR,
                            )
                        nc.vector.tensor_copy(
                            v_mine[:, t, 4 * g : 4 * g + 4, 0:D],
                            pv.rearrange("p (h d) -> p h d", d=D),
                        )
                    nc.sync.dma_start(
                        kv_in[KSZ:].rearrange(
                            "(p a b c) -> p a b c", p=P, a=4, b=4, c=VW
                        ),
                        v_mine[:, :, 4 * g : 4 * g + 4, :],
                    )

                def ag(in_ap, out_ap):
                    nc.gpsimd.collective_compute(
                        "AllGather",
                        mybir.AluOpType.bypass,
                        replica_groups=groups,
                        ins=[in_ap],
                        outs=[out_ap],
                    )

                if split:
                    do_k()
                    ag(kv_in[:KSZ].opt(), kv_out[:, :KSZ].opt())
                    do_v()
                    ag(kv_in[KSZ:].opt(), kv_out[:, KSZ:].opt())
                else:
                    do_v()
                    do_k()
                    ag(kv_in.opt(), kv_out.opt())

            def ca_rep_group(g, Wk_t, Wv_t):
                """Replicated CA kv for quad g: every core computes k/v for
                all SRCN src tokens locally (no AllGather -> no CC-init
                latency before the first CA attention group)."""
                wk = wpool.tile([P, 2, KOP, 2, P], fp8, tag="wk2", name="wkr")
                nc.sync.dma_start(
                    wk[:],
                    Wk_t[2 * g : 2 * g + 2].rearrange("h p a b m -> p h a b m"),
                )
                wv = wpool.tile([P, KOP, 2, 256], fp8, tag="wv", name="wvr")
                nc.sync.dma_start(
                    wv[:], Wv_t[:, :, :, g * 256 : (g + 1) * 256]
                )
                for c4 in range(4):
                    srcC = work.tile(
                        [P, KO, TOK], fp8, tag="srcC", bufs=2, name="srcC"
                    )
                    nc.sync.dma_start(
                        srcC[:], srcT[:, :, c4 * TOK : (c4 + 1) * TOK]
                    )
                    ckc = work.tile(
                        [P, 2, TOK], bf16, tag="ckc", bufs=1, name="ckc"
                    )
                    nc.sync.dma_start(
                        ckc[:], ckf_c[:, :, c4 * TOK : (c4 + 1) * TOK]
                    )
                    cks = work.tile(
                        [P, 2, TOK], bf16, tag="cks", bufs=1, name="cks"
                    )
                    nc.sync.dma_start(
                        cks[:], ckf_s[:, :, c4 * TOK : (c4 + 1) * TOK]
                    )
                    pq = big_ps("pqk")
                    proj_quad(pq, wk, srcC)
                    norm_rope_quad(
                        pq, ckc[:], cks[:],
                        k_full[:, g, :, c4 * TOK : (c4 + 1) * TOK],
                    )
                    # v for this chunk right away (cheap; frees srcC slot)
                    for t in range(4):
                        pvt = small_ps("pv")
                        pv = pvt[:, 0, 0:256]
                        for kp in range(KOP):
                            nc.tensor.matmul(
                                pv,
                                srcC[:, 2 * kp : 2 * kp + 2, t * P : (t + 1) * P],
                                wv[:, kp],
                                start=(kp == 0),
                                stop=(kp == KOP - 1),
                                perf_mode=DR,
                            )
                        nc.vector.tensor_copy(
                            v_full[:, 4 * c4 + t, 4 * g : 4 * g + 4, 0:D],
                            pv.rearrange("p (h d) -> p h d", d=D),
                        )

            def scatter_group(g, kv_out):
                for r in range(NR):
                    nc.sync.dma_start(
                        k_full[:, g, :, r * TOK : (r + 1) * TOK],
                        kv_out[r, :KSZ].rearrange(
                            "(p j t) -> p j t", p=P, j=2, t=TOK
                        ),
                    )
                    nc.sync.dma_start(
                        v_full[:, r * 4 : (r + 1) * 4, 4 * g : 4 * g + 4, :],
                        kv_out[r, KSZ:].rearrange(
                            "(p a b c) -> p a b c", p=P, a=4, b=4, c=VW
                        ),
                    )

            def proj_q(Wt, tabname, quads=range(HG)):
                """y -> q (given quads), normed+roped into q4."""
                for g in quads:
                    wq = wpool.tile([P, 2, KOP, 2, P], fp8, tag="wk2", name="wq")
                    nc.sync.dma_start(
                        wq[:],
                        Wt[2 * g : 2 * g + 2].rearrange("h p a b m -> p h a b m"),
                    )
                    pq = big_ps("pq")
                    proj_quad(pq, wq, yT)
                    cs, ss = tabs_sb[tabname]
                    norm_rope_quad(pq, cs[:], ss[:], q4[:, g])

            def attention_group(hg, kdb):
                """scores+softmax+AV for quad hg (pairs j=0,1); fills
                xT[:, 2hg:2hg+2] with x_norm/8 (fp8)."""
                xraw = stat.tile(
                    [P, 2, TOK], bf16, tag="xraw", bufs=1, name="xraw"
                )
                dens4 = work.tile(
                    [P, 4, TOK], bf16, tag="dens", bufs=1, name="dens4"
                )
                for j in range(2):
                    px = psum.tile(
                        [VW, 2, TOK], f32, tag="px", bufs=1, name="px"
                    )
                    for kc in range(SKC):
                        ps = big_ps("ps")
                        for i in range(2):
                            b = 2 * j + i
                            nc.tensor.matmul(
                                ps[:, i, :],
                                k_full[
                                    32 * b : 32 * b + 32,
                                    hg,
                                    :,
                                    kc * P : (kc + 1) * P,
                                ],
                                q4[32 * b : 32 * b + 32, hg],
                                start=True,
                                stop=True,
                                perf_mode=DR,
                                tile_position=(32 * b, 0),
                            )
                        if kc % 2 == 0:
                            prob = probp.tile(
                                [P, 2, 2, TOK], fp8, tag="prob", name="prob"
                            )
                        if kc in DVE_EXP_KC:
                            nc.vector.tensor_scalar(
                                prob[:, kc % 2].bitcast(u8), ps[:],
                                EXP_S1, EXP_S2, MUL, ADD,
                            )
                        else:
                            nc.scalar.activation(
                                prob[:, kc % 2],
                                ps[:],
                                AF.Exp,
                                scale=1.0 / math.sqrt(D),
                                bias=bm3_sb[:],
                            )
                        if kc % 2 == 1:
                            j2 = kc - 1
                            for i in range(2):
                                nc.tensor.matmul(
                                    px[:, i, :],
                                    v_full[:, j2 : j2 + 2, hg * 4 + 2 * j + i, :],
                                    prob[:, :, i, :],
                                    start=(kc == 1),
                                    stop=(kc == SKC - 1),
                                    perf_mode=DR,
                                )
                    for i in range(2):
                        # denom row rides on partition 64 (ones column of v)
                        nc.vector.tensor_copy(
                            dens4[D : D + 1, 2 * j + i], px[D : D + 1, i, :]
                        )
                        nc.vector.tensor_copy(
                            xraw[i * D : (i + 1) * D, j], px[0:D, i, :]
                        )
                # reciprocals: 1/(64*den); the 64 unscales v and Wo x8 each,
                # making xT = x_norm/8 which Wo's x8 restores
                nc.sync.dma_start(
                    kdb[: 4 * TOK].rearrange("(o f t) -> o f t", o=1, f=4),
                    dens4[D : D + 1],
                )
                d4 = stat.tile([4, TOK], bf16, tag="d4", bufs=1, name="d4")
                nc.sync.dma_start(
                    d4[:], kdb[: 4 * TOK].rearrange("(f t) -> f t", f=4)
                )
                nc.scalar.activation(d4[:], d4[:], AF.Ln)
                rec4 = stat.tile([4, TOK], bf16, tag="rec4", bufs=1, name="rec4")
                nc.scalar.activation(
                    rec4[:], d4[:], AF.Exp, scale=-1.0, bias=bln64_sb[:4]
                )
                nc.sync.dma_start(
                    kdb[4 * TOK :].rearrange("(f t) -> f t", f=4), rec4[:]
                )
                rec_bc = work.tile(
                    [P, 2, TOK], bf16, tag="recbc", bufs=1, name="rec_bc"
                )
                for i in range(2):
                    src = bass.AP(
                        tensor=kdb.tensor,
                        offset=kdb.offset + 4 * TOK + i * TOK,
                        ap=[[0, D], [2 * TOK, 2], [1, TOK]],
                    )
                    nc.sync.dma_start(rec_bc[i * D : (i + 1) * D], src)
                for j in range(2):
                    nc.gpsimd.tensor_mul(
                        xT[:, 2 * hg + j], xraw[:, j], rec_bc[:, j]
                    )

            def wo_group(hg, Wo_t):
                """Wo partial for head-quad hg, accumulated into resid."""
                wo = wpool.tile([P, KO, 2, P], fp8, tag="wo", name="wo")
                nc.sync.dma_start(
                    wo[:], Wo_t[:, :, hg].rearrange("o p b m -> p o b m")
                )
                for oc in range(KO):
                    pot = small_ps("po")
                    po = pot[:, 0, :]
                    nc.tensor.matmul(
                        po,
                        wo[:, oc],
                        xT[:, 2 * hg : 2 * hg + 2, :],
                        start=True,
                        stop=True,
                        perf_mode=DR,
                    )
                    nc.vector.tensor_add(resid[:, oc], resid[:, oc], po)

            def rmsnorm_feat(dst):
                """resid f32 -> dst (fp8 or bf16): resid * rsqrt(mean sq)."""
                nb = big_ps("ynb")  # ssq in bank 0, bc broadcast in bank 1
                ssq = nb[0:1, 0, :]
                for c in range(KO):
                    sq = work.tile([P, TOK], bf16, tag="ysq", name="ynsq")
                    nc.vector.tensor_mul(sq[:], resid[:, c], resid[:, c])
                    nc.tensor.matmul(
                        ssq,
                        ones_c_sb[:],
                        sq[:],
                        start=(c == 0),
                        stop=(c == KO - 1),
                    )
                lnt = stat.tile([1, TOK], bf16, tag="lnt", name="ylnt")
                nc.scalar.activation(
                    lnt[:], ssq, AF.Ln, bias=eps_sb[:1], scale=1.0 / DIM
                )
                rs = stat.tile([1, TOK], bf16, tag="rs", name="yrs")
                nc.scalar.activation(rs[:], lnt[:], AF.Exp, scale=-0.5)
                bc = nb[:, 1, :]
                nc.tensor.matmul(bc, ones_r128_sb[:], rs[:], start=True, stop=True)
                for c in range(KO):
                    nc.vector.tensor_mul(dst[:, c], resid[:, c], bc)

            # ================= cross-attention =================
            # quad 0's kv is computed locally on every core (hides the CC
            # init latency); quads 1-3 go through pipelined AllGathers.
            kvi_ca = [
                dram.tile([KSZ + VSZ], fp8, tag=f"kvica{g}", name=f"kvica{g}")
                for g in range(1, HG)
            ]
            kvo_ca = [
                dram.tile([NR, KSZ + VSZ], fp8, tag=f"kvoca{g}", name=f"kvoca{g}")
                for g in range(1, HG)
            ]
            kdbs = [
                dram.tile([8 * TOK], bf16, tag=f"kdb{g}", name=f"kdb{g}")
                for g in range(HG)
            ]
            nc.sync.dma_start(ones_c_sb[:], ones_c[:])
            nc.sync.dma_start(ones_r128_sb[:], ones_r128[:])
            load_tab("cqca")
            rmsnorm_feat(yT)
            proj_q(caWq, "cqca")
            ca_rep_group(0, caWk, caWv)
            kv_group_and_ag(1, None, caWk, caWv, "ckca", kvi_ca[0], kvo_ca[0])
            load_tab("cqsa")
            load_tab("cksa")
            attention_group(0, kdbs[0])
            kv_group_and_ag(2, None, caWk, caWv, "ckca", kvi_ca[1], kvo_ca[1])
            scatter_group(1, kvo_ca[0])
            attention_group(1, kdbs[1])
            wo_group(0, caWo)
            kv_group_and_ag(3, None, caWk, caWv, "ckca", kvi_ca[2], kvo_ca[2])
            scatter_group(2, kvo_ca[1])
            attention_group(2, kdbs[2])
            wo_group(1, caWo)
            scatter_group(3, kvo_ca[2])
            attention_group(3, kdbs[3])
            wo_group(2, caWo)
            wo_group(3, caWo)

            # ================= self-attention =================
            rmsnorm_feat(yT)
            kvi_sa = [
                dram.tile([KSZ + VSZ], fp8, tag=f"kvisa{g}", name=f"kvisa{g}")
                for g in range(HG)
            ]
            kvo_sa = [
                dram.tile([NR, KSZ + VSZ], fp8, tag=f"kvosa{g}", name=f"kvosa{g}")
                for g in range(HG)
            ]
            kdbs2 = [
                dram.tile([8 * TOK], bf16, tag=f"kdc{g}", name=f"kdc{g}")
                for g in range(HG)
            ]
            for g in range(HG):
                kv_group_and_ag(
                    g, yT, saWk, saWv, "cksa", kvi_sa[g], kvo_sa[g]
                )
                if g == 0:
                    proj_q(saWq, "cqsa")
            for hg in range(HG):
                scatter_group(hg, kvo_sa[hg])
                attention_group(hg, kdbs2[hg])
                if hg >= 1:
                    wo_group(hg - 1, saWo)
            wo_group(HG - 1, saWo)

            # ================= FFN (bf16) =================
            rmsnorm_feat(yF)
            for qtr in range(4):
                hT = htp.tile([P, 8, TOK], bf16, tag="hT", name="hT")
                for e in range(4):
                    w1 = w13p.tile([P, 2, KO, P], bf16, tag="w1", name="w1")
                    nc.sync.dma_start(
                        w1[:],
                        W1i[qtr * 8 + e * 2 : qtr * 8 + e * 2 + 2].rearrange(
                            "h p a m -> p h a m"
                        ),
                    )
                    w3 = w13p.tile([P, 2, KO, P], bf16, tag="w3", name="w3")
                    nc.sync.dma_start(
                        w3[:],
                        W3i[qtr * 8 + e * 2 : qtr * 8 + e * 2 + 2].rearrange(
                            "h p a m -> p h a m"
                        ),
                    )
                    for gg in range(2):
                        p13 = big_ps("p13")
                        p1 = p13[:, 0, :]
                        p3 = p13[:, 1, :]
                        for c in range(KO):
                            nc.tensor.matmul(
                                p1, w1[:, gg, c], yF[:, c],
                                start=(c == 0), stop=(c == KO - 1),
                            )
                        for c in range(KO):
                            nc.tensor.matmul(
                                p3, w3[:, gg, c], yF[:, c],
                                start=(c == 0), stop=(c == KO - 1),
                            )
                        s1 = stat.tile([P, TOK], bf16, tag="s1", name="s1")
                        nc.scalar.activation(s1[:], p1, AF.Silu)
                        nc.vector.tensor_mul(hT[:, e * 2 + gg], s1[:], p3)
                w2 = w2p.tile([P, KO, 8, P], bf16, tag="w2", name="w2")
                nc.sync.dma_start(
                    w2[:],
                    W2i[:, :, qtr * 8 : (qtr + 1) * 8].rearrange(
                        "o p a m -> p o a m"
                    ),
                )
                for oc in range(KO):
                    pot = small_ps("po2")
                    po = pot[:, 0, :]
                    for gg in range(8):
                        nc.tensor.matmul(
                            po, w2[:, oc, gg], hT[:, gg],
                            start=(gg == 0), stop=(gg == 7),
                        )
                    nc.vector.tensor_add(resid[:, oc], resid[:, oc], po)
                    if qtr == 3:
                        nc.sync.dma_start(outT[:, oc], resid[:, oc])

    _split_multiwait(nc)
    return nc


def _prep_inputs(inputs):
    """Full problem inputs -> list of 8 per-core in_maps."""
    tgt = np.asarray(inputs["tgt"], np.float32)
    src = np.asarray(inputs["src"], np.float32)
    tgt_pos = np.asarray(inputs["tgt_pos"], np.int32)
    src_pos = np.asarray(inputs["src_pos"], np.int32)

    pre_ca_w = np.asarray(inputs["pre_ca_w"], np.float32)
    pre_sa_w = np.asarray(inputs["pre_sa_w"], np.float32)
    pre_ffn_w = np.asarray(inputs["pre_ffn_w"], np.float32)

    def fold(Wname, w):
        return np.asarray(inputs[Wname], np.float32) * w[:, None]

    ca_Wq = fold("ca_Wq", pre_ca_w)
    ca_Wkv = np.asarray(inputs["ca_Wkv"], np.float32)
    ca_Wk, ca_Wv = ca_Wkv[:, :DIM], ca_Wkv[:, DIM:]
    ca_Wo = np.asarray(inputs["ca_Wo"], np.float32)
    sa_Wq = fold("sa_Wq", pre_sa_w)
    sa_Wkv = fold("sa_Wkv", pre_sa_w)
    sa_Wk, sa_Wv = sa_Wkv[:, :DIM], sa_Wkv[:, DIM:]
    sa_Wo = np.asarray(inputs["sa_Wo"], np.float32)
    W1 = fold("W1", pre_ffn_w)
    W3 = fold("W3", pre_ffn_w)
    W2 = np.asarray(inputs["W2"], np.float32)

    shared = {
        "caWq": _lhsT_dr(ca_Wq[:, _QPERM]),
        "caWk": _lhsT_dr(ca_Wk[:, _QPERM]),
        "caWv": _rhs_dr(ca_Wv),
        "caWo": _lhsT_dr(ca_Wo),
        "saWq": _lhsT_dr(sa_Wq[:, _QPERM]),
        "saWk": _lhsT_dr(sa_Wk[:, _QPERM]),
        "saWv": _rhs_dr(sa_Wv),
        "saWo": _lhsT_dr(sa_Wo),
        "W1": _lhsT_bf(W1),
        "W3": _lhsT_bf(W3),
        "W2": _lhsT_bf(W2),
    }

    blk4 = np.zeros((P, 4), np.float32)
    for m in range(4):
        blk4[32 * m : 32 * m + 32, m] = 1
    shared["blk4"] = blk4.astype(BF).copy()
    shared["mask4"] = blk4.T.astype(BF).copy()
    shared["ones_c"] = np.ones((P, 1), BF)
    shared["ones_r128"] = np.ones((1, P), BF)

    ca_qn = np.asarray(inputs["ca_qn"], np.float32)
    ca_kn = np.asarray(inputs["ca_kn"], np.float32)
    sa_qn = np.asarray(inputs["sa_qn"], np.float32)
    sa_kn = np.asarray(inputs["sa_kn"], np.float32)

    srcT_full = [_featmajor(src[s], F8) for s in range(B)]
    ckf = [_rope_tables_quad(src_pos[s], ca_kn) for s in range(B)]

    in_maps = []
    for c in range(NCORES):
        s, part = c // NR, c % NR
        rows = slice(part * TOK, (part + 1) * TOK)
        m = dict(shared)
        m["tgtT"] = _featmajor(tgt[s, rows], np.float32)
        m["srcT"] = srcT_full[s]
        m["srcTm"] = _featmajor(src[s, rows], F8)
        m["ckf_c"], m["ckf_s"] = ckf[s]
        tpos = tgt_pos[s, rows]
        spos = src_pos[s, rows]
        for tn, (pos, nv) in {
            "cqca": (tpos, ca_qn),
            "ckca": (spos, ca_kn),
            "cqsa": (tpos, sa_qn),
            "cksa": (tpos, sa_kn),
        }.items():
            ct, st = _rope_tables_quad(pos, nv)
            m[tn + "_c"] = ct
            m[tn + "_s"] = st
        in_maps.append(m)
    return in_maps


def _get_nc():
    if "nc" not in _cache:
        _cache["nc"] = _build_bass()
    return _cache["nc"]


def run(inputs, trace=False):
    """Run on 8 cores; returns (full_output, exec_time_ns_or_None)."""
    if trace:
        _install_ntff_hook()
    from concourse.bass_utils import run_bass_kernel_spmd

    in_maps = _prep_inputs(inputs)
    nc = _get_nc()
    res = run_bass_kernel_spmd(
        nc, in_maps, core_ids=list(range(NCORES)), trace=trace
    )
    out = np.empty((B, N, DIM), np.float32)
    for c in range(NCORES):
        s, part = c // NR, c % NR
        arr = np.asarray(res.results[c]["outT"])  # [128, 8, TOK]
        rows = slice(part * TOK, (part + 1) * TOK)
        out[s, rows] = np.transpose(arr, (2, 1, 0)).reshape(TOK, DIM)
    return out, res.exec_time_ns


def kernel(**inputs):
    out, _ = run(inputs, trace=False)
    return out
